# revision 6
# baseline (speedup 1.0000x reference)
"""Trainium2 Bass kernel for a GPT-2 style transformer block (pre-LN, no mask).

Reference shapes: x [B=2, T=2048, C=1024], H=16 heads, MLP hidden 4C=4096.

Sharding (8 NeuronCores): data-parallel over B (cores 0-3 -> batch 0,
cores 4-7 -> batch 1); within each 4-core group the 2048 rows are split
512 per core. Each core computes LN1 + Q/K/V only for its OWN 512 rows,
then K^T and V are AllGather'd across the 4-core group (K in 4 chunks of
2 head-pairs, V in 2 feature halves) so attention/proj/MLP stay fully
local per core. A tiny dummy AllGather issued at t=0 absorbs the
first-collective latency (ncfw warmup / core-start skew).

Compute layout: activations feeding matmul contractions are kept
feature-major ("transposed", [C, t]) via the DMA xbar transpose; scores
are computed as S^T = K Q^T per head ([tk, tq]) with two heads packed
into the 128-wide contraction via row tiling (the two matmuls run
concurrently in disjoint PE row groups); exp runs on the scalar engine
straight out of PSUM; P @ V uses a [V | ones] stationary operand so the
softmax denominators accumulate in the same PSUM tile as Y^T.

LayerNorm rstd is computed as exp(-0.5*ln(var+eps)) so the scalar
engine only ever needs the natural_log_exp table set plus gelu -- two
table loads total, both off the critical path (a dummy gelu during the
proj phase prefetches the gelu set).

Weights are pre-tiled on the host into the exact SBUF layouts so every
DMA is a contiguous 128-partition transfer.
"""

import numpy as np
import ml_dtypes

import concourse.bass as bass
import concourse.bacc as bacc
import concourse.tile as tile
from concourse import mybir
from concourse.bass import ts, ds
from concourse.bass_utils import run_bass_kernel_spmd

f32 = mybir.dt.float32
bf16 = mybir.dt.bfloat16
AF = mybir.ActivationFunctionType
OP = mybir.AluOpType

B, T, C, H = 2, 2048, 1024, 16
DH = C // H          # 64
F = 4 * C            # 4096
NCORES = 8
GROUP = 4            # cores per batch
TQ = T // GROUP      # 512 own rows per core
NT = T // 128        # 16 key tiles
CCH = C // 128       # 8 contraction chunks over C
PAIRS = H // 2       # 8 head pairs
FT = F // 128        # 32 hidden tiles
QT = TQ // 128       # 4 own-row tiles
VW = DH + 1          # 65: V columns per head incl. ones column
GROUPS = [[0, 1, 2, 3], [4, 5, 6, 7]]

_CACHED = {}


def _bcast(ap, parts=128):
    """DRAM AP for a 1-D tensor broadcast across `parts` partitions."""
    return bass.AP(tensor=ap.tensor, offset=ap.offset, ap=[[0, parts]] + list(ap.ap))


def _build_program(trivial_ln1, trivial_ln2, trivial_b):
    nc = bacc.Bacc("TRN2", target_bir_lowering=False, debug=False,
                   num_devices=NCORES)

    xq = nc.dram_tensor("xq", [TQ, C], f32, kind="ExternalInput")
    # pre-tiled weights: [128 (c within chunk), CCH, out-features]
    wq = nc.dram_tensor("wq", [128, CCH, C], bf16, kind="ExternalInput")
    wk = nc.dram_tensor("wk", [128, CCH, C], bf16, kind="ExternalInput")
    wv = nc.dram_tensor("wv", [128, CCH, C], bf16, kind="ExternalInput")
    bqv = nc.dram_tensor("bq", [128, PAIRS], f32, kind="ExternalInput")
    bkv = nc.dram_tensor("bk", [128, PAIRS], f32, kind="ExternalInput")
    bvv = nc.dram_tensor("bv", [C], f32, kind="ExternalInput")
    ln1w = nc.dram_tensor("ln1w", [C], f32, kind="ExternalInput")
    ln1b = nc.dram_tensor("ln1b", [C], f32, kind="ExternalInput")
    ln2w = nc.dram_tensor("ln2w", [C], f32, kind="ExternalInput")
    ln2b = nc.dram_tensor("ln2b", [C], f32, kind="ExternalInput")
    wp = nc.dram_tensor("wp", [128, CCH, C], bf16, kind="ExternalInput")
    bp = nc.dram_tensor("bp", [C], f32, kind="ExternalInput")
    # wf pre-tiled per f'-tile: [FT, 128 (c), CCH, 128 (f')]
    wf = nc.dram_tensor("wf", [FT, 128, CCH, 128], bf16, kind="ExternalInput")
    bf_ = nc.dram_tensor("bf", [128, FT], f32, kind="ExternalInput")
    wm = nc.dram_tensor("wm", [F, C], bf16, kind="ExternalInput")
    bm = nc.dram_tensor("bm", [C], f32, kind="ExternalInput")
    out = nc.dram_tensor("out", [TQ, C], f32, kind="ExternalOutput")

    # collective scratch (Internal DRAM)
    ko = [nc.dram_tensor(f"ko{a}", [256, TQ], bf16, kind="Internal")
          for a in range(4)]
    kg = [nc.dram_tensor(f"kg{a}", [GROUP, 256, TQ], bf16, kind="Internal")
          for a in range(4)]
    vo = [nc.dram_tensor(f"vo{h}", [TQ, 8 * VW], bf16, kind="Internal")
          for h in range(2)]
    vg = [nc.dram_tensor(f"vg{h}", [GROUP, TQ, 8 * VW], bf16, kind="Internal")
          for h in range(2)]
    din = nc.dram_tensor("din", [64], bf16, kind="Internal")
    dout = nc.dram_tensor("dout", [GROUP * 64], bf16, kind="Internal")

    with tile.TileContext(nc) as tc:
        _emit(nc, tc, trivial_ln1, trivial_ln2, trivial_b,
              xq, wq, wk, wv, bqv, bkv, bvv, ln1w, ln1b, ln2w, ln2b,
              wp, bp, wf, bf_, wm, bm, out,
              ko, kg, vo, vg, din, dout)
    nc.compile()
    return nc


def _emit(nc, tc, trivial_ln1, trivial_ln2, trivial_b,
          xq, wq, wk, wv, bqv, bkv, bvv, ln1w, ln1b, ln2w, ln2b,
          wp, bp, wf, bf_, wm, bm, out,
          ko, kg, vo, vg, din, dout):
    from contextlib import ExitStack

    with ExitStack() as st:
        persist = st.enter_context(tc.tile_pool(name="persist", bufs=1))
        stat = st.enter_context(tc.tile_pool(name="stat", bufs=4))
        stream = st.enter_context(tc.tile_pool(name="stream", bufs=4))

        eps_t = persist.tile([128, 1], f32)
        nc.vector.memset(eps_t, 1e-5)

        # ---- dummy warm-up collective: absorbs ncfw boot / core skew ----
        d_t = persist.tile([1, 64], bf16)
        nc.vector.memset(d_t, 1.0)
        nc.sync.dma_start(out=din.ap().rearrange("(p c) -> p c", p=1), in_=d_t)
        nc.gpsimd.collective_compute(
            "AllGather", OP.bypass, replica_groups=GROUPS,
            ins=[din.ap().opt()], outs=[dout.ap().opt()])

        def layer_norm(x_t, w_bc, b_bc, out_ap, trivial):
            """x_t [128, C] f32 -> out_ap [128, C] bf16 (normalized + affine).

            rstd = exp(-0.5 * ln(var + eps)) keeps ACT on the ln/exp table."""
            stats = stat.tile([128, 2, nc.vector.BN_STATS_DIM], f32, name="stats", bufs=6)
            nc.vector.bn_stats(out=stats[:, 0, :], in_=x_t[:, 0:512])
            nc.vector.bn_stats(out=stats[:, 1, :], in_=x_t[:, 512:1024])
            mv = stat.tile([128, nc.vector.BN_AGGR_DIM], f32, name="mv", bufs=6)
            nc.vector.bn_aggr(out=mv, in_=stats)
            lnv = stat.tile([128, 1], f32, name="lnv", bufs=6)
            nc.scalar.activation(lnv, mv[:, 1:2], AF.Ln, bias=eps_t)
            rstd = stat.tile([128, 1], f32, name="rstd", bufs=6)
            nc.scalar.activation(rstd, lnv, AF.Exp, scale=-0.5)
            if trivial:
                nc.vector.tensor_scalar(out=out_ap, in0=x_t, scalar1=mv[:, 0:1],
                                        scalar2=rstd, op0=OP.subtract, op1=OP.mult)
            else:
                t1 = stat.tile([128, C], f32, name="t1", tag="ln_t1")
                nc.vector.tensor_scalar(out=t1, in0=x_t, scalar1=mv[:, 0:1],
                                        scalar2=rstd, op0=OP.subtract, op1=OP.mult)
                nc.vector.tensor_mul(t1, t1, w_bc)
                nc.vector.tensor_add(out_ap, t1, b_bc)

        # ---------------- pools ----------------
        stA = st.enter_context(ExitStack())
        pA = stA.enter_context(tc.tile_pool(name="pA", bufs=1, side="left"))
        pR = st.enter_context(tc.tile_pool(name="pR", bufs=1, side="right"))

        wk_sb = pA.tile([128, CCH, C], bf16)
        nc.sync.dma_start(out=wk_sb, in_=wk.ap())
        if not trivial_b:
            bq_sb = pA.tile([128, PAIRS], f32)
            nc.sync.dma_start(out=bq_sb, in_=bqv.ap())
            bk_sb = pA.tile([128, PAIRS], f32)
            nc.sync.dma_start(out=bk_sb, in_=bkv.ap())
            bv_bc = pA.tile([128, C], f32)
            nc.sync.dma_start(out=bv_bc, in_=_bcast(bvv.ap()))
        if not trivial_ln1:
            ln1w_bc = pA.tile([128, C], f32)
            nc.sync.dma_start(out=ln1w_bc, in_=_bcast(ln1w.ap()))
            ln1b_bc = pA.tile([128, C], f32)
            nc.sync.dma_start(out=ln1b_bc, in_=_bcast(ln1b.ap()))
        else:
            ln1w_bc = ln1b_bc = None
        wq_sb = pA.tile([128, CCH, C], bf16)
        nc.sync.dma_start(out=wq_sb, in_=wq.ap())
        wv_sb = pA.tile([128, CCH, C], bf16)
        nc.sync.dma_start(out=wv_sb, in_=wv.ap())

        # persistent activations
        hT = pA.tile([128, QT, CCH, 128], bf16)
        xqs = pR.tile([128, QT, C], f32)          # own x rows (LN1 + residual)
        qT = pR.tile([128, PAIRS, TQ], bf16)
        v_sb = pR.tile([128, NT, 8 * VW * 2], bf16)   # [tok, tile, 16*(DH+1)]
        ynT = pR.tile([128, PAIRS, TQ], bf16)

        # ---- LN1 over own 4 tiles ----
        for i in range(QT):
            nc.sync.dma_start(out=xqs[:, i], in_=xq.ap()[ts(i, 128), :])
            h_t = stream.tile([128, C], bf16, name="h_t", tag="h_t", bufs=6)
            layer_norm(xqs[:, i], ln1w_bc, ln1b_bc, h_t, trivial_ln1)
            nc.sync.dma_start_transpose(hT[:, i], h_t[:])

        # ---- own K/Q (per 2-pair chunk) + V halves, AllGathers fired ASAP ----
        kvst = st.enter_context(ExitStack())
        kv_ps = kvst.enter_context(tc.tile_pool(name="kv_ps", bufs=2, space="PSUM"))
        vos_pool = kvst.enter_context(tc.tile_pool(name="vos", bufs=2))

        def k_chunk(a, on_act):
            for e in range(2):
                j = 2 * a + e
                ps = kv_ps.tile([128, TQ], f32, name="ps_k", tag="ps_kv")
                for c in range(CCH):
                    nc.tensor.matmul(ps, wk_sb[:, c, ts(j, 128)],
                                     hT[:, 0:QT, c, :],
                                     start=(c == 0), stop=(c == CCH - 1))
                kt_t = stream.tile([128, TQ], bf16, name="kt_t", tag="kt_t", bufs=4)
                if trivial_b:
                    if on_act:
                        nc.scalar.activation(kt_t, ps, AF.Identity)
                    else:
                        nc.vector.tensor_copy(kt_t, ps)
                else:
                    nc.vector.tensor_scalar(out=kt_t, in0=ps,
                                            scalar1=bk_sb[:, j:j + 1],
                                            scalar2=None, op0=OP.add)
                nc.sync.dma_start(out=ko[a].ap()[ts(e, 128), :], in_=kt_t)
            nc.gpsimd.collective_compute(
                "AllGather", OP.bypass, replica_groups=GROUPS,
                ins=[ko[a].ap().opt()], outs=[kg[a].ap().opt()])

        def q_chunk(a, on_act):
            for e in range(2):
                j = 2 * a + e
                ps = kv_ps.tile([128, TQ], f32, name="ps_q", tag="ps_kv")
                for c in range(CCH):
                    nc.tensor.matmul(ps, wq_sb[:, c, ts(j, 128)],
                                     hT[:, 0:QT, c, :],
                                     start=(c == 0), stop=(c == CCH - 1))
                if trivial_b:
                    if on_act:
                        nc.scalar.activation(qT[:, j], ps, AF.Identity)
                    else:
                        nc.vector.tensor_copy(qT[:, j], ps)
                else:
                    nc.vector.tensor_scalar(out=qT[:, j], in0=ps,
                                            scalar1=bq_sb[:, j:j + 1],
                                            scalar2=None, op0=OP.add)

        def v_half(h, on_act):
            vos = vos_pool.tile([128, QT, 8 * VW], bf16, name="vos", tag="vos")
            ones_view = vos.rearrange("p q (h x) -> p q h x", x=VW)[:, :, :, DH:DH + 1]
            nc.vector.memset(ones_view, 1.0)
            for i in range(QT):
                ps = kv_ps.tile([128, 512], f32, name="ps_v", tag="ps_kv")
                for c in range(CCH):
                    nc.tensor.matmul(ps, hT[:, i, c, :],
                                     wv_sb[:, c, ds(512 * h, 512)],
                                     start=(c == 0), stop=(c == CCH - 1))
                dst = vos.rearrange("p q (h x) -> p q h x", x=VW)[:, i, :, 0:DH]
                psv = ps.rearrange("p (h x) -> p h x", x=DH)
                if trivial_b:
                    if on_act:
                        nc.scalar.activation(dst, psv, AF.Identity)
                    else:
                        nc.vector.tensor_copy(dst, psv)
                else:
                    bvw = bv_bc[:, ds(512 * h, 512)].rearrange(
                        "p (h x) -> p h x", x=DH)
                    nc.vector.tensor_add(dst, psv, bvw)
            nc.sync.dma_start(
                out=vo[h].ap().rearrange("(q p) f -> p q f", p=128), in_=vos)
            nc.gpsimd.collective_compute(
                "AllGather", OP.bypass, replica_groups=GROUPS,
                ins=[vo[h].ap().opt()], outs=[vg[h].ap().opt()])

        # chunk 0 early (copies on ACT, which is idle before attention);
        # later chunks' PSUM->SBUF copies go on DVE to keep ACT free for exp
        k_chunk(0, True)
        q_chunk(0, True)
        v_half(0, True)

        def v_in(h):
            # gathered V -> resident v_sb, per key tile
            for t in range(NT):
                r, w = t // 4, t % 4
                nc.sync.dma_start(
                    out=v_sb[:, t, ds(8 * VW * h, 8 * VW)],
                    in_=vg[h].ap()[r, ts(w, 128), :])

        # ---- attention (baseline structure; kT comes from the AllGather) ----
        scale = 1.0 / float(np.sqrt(DH))
        att_ctx = st.enter_context(ExitStack())
        s_ps = att_ctx.enter_context(tc.tile_pool(name="s_ps", bufs=2, space="PSUM"))
        y_ps = att_ctx.enter_context(tc.tile_pool(name="y_ps", bufs=1, space="PSUM"))
        kq_sb = att_ctx.enter_context(tc.tile_pool(name="kq_sb", bufs=2))
        att_sb = att_ctx.enter_context(tc.tile_pool(name="att_sb", bufs=3))

        def attention_pair(j):
            a, e = j // 2, j % 2
            kT_j = kq_sb.tile([128, T], bf16, name="kT_j", tag="kT_j")
            for r in range(GROUP):
                nc.sync.dma_start(out=kT_j[:, ds(TQ * r, TQ)],
                                  in_=kg[a].ap()[r, ts(e, 128), :])
            ps_y1 = y_ps.tile([VW, 512], f32, name="ps_y1", tag="ps_y1")
            ps_y2 = y_ps.tile([VW, 512], f32, name="ps_y2", tag="ps_y2")
            for cidx in range(NT):
                ps_s = s_ps.tile([128, 1024], f32, name="ps_s", tag="ps_s")
                nc.tensor.matmul(ps_s[:, 0:512],
                                 kT_j[0:64, ts(cidx, 128)],
                                 qT[0:64, j, :], start=True, stop=True)
                nc.tensor.matmul(ps_s[:, 512:1024],
                                 kT_j[64:128, ts(cidx, 128)],
                                 qT[64:128, j, :], start=True, stop=True,
                                 tile_position=(64, 0))
                pT = att_sb.tile([128, 1024], bf16, name="pT", tag="pT")
                nc.scalar.activation(pT, ps_s, AF.Exp, scale=scale)
                nc.tensor.matmul(ps_y1, v_sb[:, cidx, ds(VW * 2 * j, VW)],
                                 pT[:, 0:512],
                                 start=(cidx == 0), stop=(cidx == NT - 1))
                nc.tensor.matmul(ps_y2, v_sb[:, cidx, ds(VW * (2 * j + 1), VW)],
                                 pT[:, 512:1024],
                                 start=(cidx == 0), stop=(cidx == NT - 1))
            for u, ps_y in ((0, ps_y1), (1, ps_y2)):
                # copy Y and the sums row out of PSUM right away so the
                # accumulator banks free up for the next pair; the sums
                # staging copy also moves them to SBUF partition 0
                # (custom-DVE ops mis-read PSUM at a partition offset)
                ycp = att_sb.tile([64, 512], f32, name="ycp", tag="ycp")
                nc.vector.tensor_copy(ycp, ps_y[0:DH, :])
                rs0 = att_sb.tile([1, 512], f32, name="rs0", tag="rs0")
                nc.vector.tensor_copy(rs0, ps_y[DH:DH + 1, :])
                rs = att_sb.tile([1, 512], f32, name="rs", tag="rs")
                nc.vector.reciprocal_approx_fast(rs, rs0)
                bc = att_sb.tile([64, 512], f32, name="bc", tag="bc")
                nc.gpsimd.partition_broadcast(bc, rs)
                nc.vector.tensor_mul(ynT[64 * u:64 * u + 64, j, :],
                                     ycp, bc)

        # interleave: pair 0 right after chunk 0 so exp starts early;
        # remaining own-QKV chunks fill PE while AGs are in flight
        v_in(0)
        attention_pair(0)
        k_chunk(1, False)
        q_chunk(1, False)
        v_half(1, False)
        attention_pair(1)
        k_chunk(2, False)
        q_chunk(2, False)
        v_in(1)
        attention_pair(2)
        k_chunk(3, False)
        q_chunk(3, False)
        attention_pair(3)
        for j in range(4, PAIRS):
            attention_pair(j)
        att_ctx.close()
        kvst.close()
        stA.close()

        # wp + proj-era constants (DMA'd during attention; emitted earlier
        # so the transfers overlap, but placed in pR which persists)
        wp_sb = pR.tile([128, CCH, C], bf16)
        nc.sync.dma_start(out=wp_sb, in_=wp.ap())
        if not trivial_ln2:
            ln2w_bc = pR.tile([128, C], f32)
            nc.sync.dma_start(out=ln2w_bc, in_=_bcast(ln2w.ap()))
            ln2b_bc = pR.tile([128, C], f32)
            nc.sync.dma_start(out=ln2b_bc, in_=_bcast(ln2b.ap()))
        else:
            ln2w_bc = ln2b_bc = None
        if not trivial_b:
            bp_bc = pR.tile([128, C], f32)
            nc.sync.dma_start(out=bp_bc, in_=_bcast(bp.ap()))

        # ---- attn projection + residual + LN2 + h2^T ----
        pD = st.enter_context(tc.tile_pool(name="pD", bufs=1, side="left"))
        x2 = pD.tile([128, QT, C], f32)
        h2T = pD.tile([128, QT, CCH, 128], bf16)
        bfc_sb = pD.tile([128, FT], f32)
        nc.sync.dma_start(out=bfc_sb, in_=bf_.ap())
        bm_bc = pD.tile([128, C], f32)
        nc.sync.dma_start(out=bm_bc, in_=_bcast(bm.ap()))

        with tc.tile_pool(name="ap_ps", bufs=2, space="PSUM") as ap_ps:
            for i in range(QT):
                if not trivial_b:
                    nc.vector.tensor_add(xqs[:, i], xqs[:, i], bp_bc)
                for n in range(C // 512):
                    ps = ap_ps.tile([128, 512], f32, name="ps_a", tag="ps_a")
                    for j in range(PAIRS):
                        nc.tensor.matmul(ps, ynT[:, j, ts(i, 128)],
                                         wp_sb[:, j, ds(512 * n, 512)],
                                         start=(j == 0), stop=(j == PAIRS - 1))
                    nc.vector.tensor_add(x2[:, i, ds(512 * n, 512)], ps,
                                         xqs[:, i, ds(512 * n, 512)])
                h2_t = stream.tile([128, C], bf16, name="h2_t", tag="h_t", bufs=6)
                layer_norm(x2[:, i, :], ln2w_bc, ln2b_bc, h2_t, trivial_ln2)
                nc.sync.dma_start_transpose(h2T[:, i], h2_t[:])
                if i == QT - 1:
                    # prefetch the gelu table while proj/LN2 finish
                    gd = stat.tile([128, 1], f32, name="gd", bufs=6)
                    nc.scalar.activation(gd, eps_t, AF.Gelu_apprx_tanh)

        # ---- MLP ----
        # fold the mlp_proj bias into the residual copy while fc runs
        for i in range(QT):
            nc.vector.tensor_add(x2[:, i, :], x2[:, i, :], bm_bc)
        gT = pD.tile([128, FT, TQ], bf16)
        with tc.tile_pool(name="fc_ps", bufs=4, space="PSUM") as fc_ps, \
             tc.tile_pool(name="wf_sb", bufs=6) as wf_pool:
            for t in range(FT):
                wf_t = wf_pool.tile([128, CCH, 128], bf16, name="wf_t", tag="wf_t")
                nc.sync.dma_start(out=wf_t, in_=wf.ap()[t])
                ps = fc_ps.tile([128, 512], f32, name="ps_f", tag="ps_f")
                for c in range(CCH):
                    nc.tensor.matmul(ps, wf_t[:, c, :], h2T[:, 0:QT, c, :],
                                     start=(c == 0), stop=(c == CCH - 1))
                nc.scalar.activation(gT[:, t, :], ps, AF.Gelu_apprx_tanh,
                                     bias=bfc_sb[:, t:t + 1], scale=1.0)

        with tc.tile_pool(name="m_ps", bufs=1, space="PSUM") as m_ps, \
             tc.tile_pool(name="wm_sb", bufs=6) as wm_pool, \
             tc.tile_pool(name="out_sb", bufs=2) as out_pool:
            ps_m = [m_ps.tile([128, 512], f32, name=f"ps_m{k}", tag=f"ps_m{k}")
                    for k in range(8)]
            for t in range(FT):
                wm_t = wm_pool.tile([128, C], bf16, name="wm_t", tag="wm_t")
                nc.sync.dma_start(out=wm_t, in_=wm.ap()[ts(t, 128), :])
                for i in range(QT):
                    for n in range(C // 512):
                        nc.tensor.matmul(ps_m[i * 2 + n], gT[:, t, ts(i, 128)],
                                         wm_t[:, ds(512 * n, 512)],
                                         start=(t == 0), stop=(t == FT - 1))
            for i in range(QT):
                out_t = out_pool.tile([128, C], f32, name="out_t", tag="out_t")
                for n in range(C // 512):
                    # DMA per 512-col chunk right away so the tail drains fast
                    nc.vector.tensor_add(out_t[:, ds(512 * n, 512)],
                                         ps_m[i * 2 + n],
                                         x2[:, i, ds(512 * n, 512)])
                    nc.sync.dma_start(out=out.ap()[ts(i, 128), ds(512 * n, 512)],
                                      in_=out_t[:, ds(512 * n, 512)])


def _get_program(trivial_ln1, trivial_ln2, trivial_b):
    key = (trivial_ln1, trivial_ln2, trivial_b)
    if key not in _CACHED:
        _CACHED[key] = _build_program(trivial_ln1, trivial_ln2, trivial_b)
    return _CACHED[key]


def _tile_proj_weight(w):
    # [C, N] f32 -> [128, CCH, N] bf16 with partition = c % 128, chunk = c // 128
    w = np.asarray(w, np.float32).reshape(CCH, 128, -1)
    return np.ascontiguousarray(w.transpose(1, 0, 2).astype(ml_dtypes.bfloat16))


def _prep_in_maps(inputs):
    fl = lambda a: np.ascontiguousarray(np.asarray(a, np.float32))
    x = fl(inputs["x"])
    attn_w = fl(inputs["attn_w"])
    attn_b = fl(inputs["attn_b"])
    wf_full = fl(inputs["fc_w"])  # [C, F]
    # wf tiled: [FT, 128(c), CCH, 128(f')]
    wf_t = wf_full.reshape(CCH, 128, FT, 128).transpose(2, 1, 0, 3)
    wf_t = np.ascontiguousarray(wf_t.astype(ml_dtypes.bfloat16))
    pb = lambda b: np.ascontiguousarray(
        np.asarray(b, np.float32).reshape(-1, 128).T)  # [128, tiles]
    shared = {
        "wq": _tile_proj_weight(attn_w[:, 0:C]),
        "wk": _tile_proj_weight(attn_w[:, C:2 * C]),
        "wv": _tile_proj_weight(attn_w[:, 2 * C:3 * C]),
        "bq": pb(attn_b[0:C]), "bk": pb(attn_b[C:2 * C]),
        "bv": fl(attn_b[2 * C:3 * C]),
        "ln1w": fl(inputs["ln1_w"]), "ln1b": fl(inputs["ln1_b"]),
        "ln2w": fl(inputs["ln2_w"]), "ln2b": fl(inputs["ln2_b"]),
        "wp": _tile_proj_weight(inputs["attn_proj_w"]),
        "bp": fl(inputs["attn_proj_b"]),
        "wf": wf_t, "bf": pb(inputs["fc_b"]),
        "wm": np.ascontiguousarray(fl(inputs["mlp_proj_w"]).astype(ml_dtypes.bfloat16)),
        "bm": fl(inputs["mlp_proj_b"]),
    }
    in_maps = []
    for core in range(NCORES):
        b, r = core // GROUP, core % GROUP
        in_maps.append({
            "xq": np.ascontiguousarray(x[b, TQ * r:TQ * (r + 1)]),
            **shared,
        })
    return in_maps


def run(inputs, trace=False):
    trivial_ln1 = bool(np.all(np.asarray(inputs["ln1_w"]) == 1.0)
                       and np.all(np.asarray(inputs["ln1_b"]) == 0.0))
    trivial_ln2 = bool(np.all(np.asarray(inputs["ln2_w"]) == 1.0)
                       and np.all(np.asarray(inputs["ln2_b"]) == 0.0))
    trivial_b = bool(np.all(np.asarray(inputs["attn_b"]) == 0.0))
    nc = _get_program(trivial_ln1, trivial_ln2, trivial_b)
    in_maps = _prep_in_maps(inputs)
    res = run_bass_kernel_spmd(nc, in_maps, core_ids=list(range(NCORES)),
                               trace=trace)
    out = np.empty((B, T, C), np.float32)
    for core in range(NCORES):
        b, r = core // GROUP, core % GROUP
        out[b, TQ * r:TQ * (r + 1)] = res.results[core]["out"]
    return out, res


def kernel(**inputs):
    out, _ = run(inputs, trace=False)
    return out


# revision 8
# speedup vs baseline: 1.0125x; 1.0125x over previous
"""Trainium2 Bass kernel for a GPT-2 style transformer block (pre-LN, no mask).

Reference shapes: x [B=2, T=2048, C=1024], H=16 heads, MLP hidden 4C=4096.

Sharding (8 NeuronCores): data-parallel over B (cores 0-3 -> batch 0,
cores 4-7 -> batch 1); within each 4-core group the 2048 rows are split
512 per core. Each core computes LN1 + Q/K/V only for its OWN 512 rows,
then K^T and V are AllGather'd across the 4-core group in fp8e4 (2-pair
chunks, K and V alternating) so attention/proj/MLP stay fully local per
core. A tiny dummy AllGather issued at t=0 absorbs the first-collective
latency (ncfw warmup / core-start skew); its input DMA is issued from
the gpsimd queue so it cannot serialize the sync-engine DMA stream.

Compute layout: activations feeding matmul contractions are kept
feature-major ("transposed", [C, t]) via the DMA xbar transpose; scores
are computed as S^T = K Q^T per head ([tk, tq]) with two heads packed
into the 128-wide contraction via row tiling (the two matmuls run
concurrently in disjoint PE row groups); exp runs on the scalar engine
straight out of PSUM; P @ V uses a [V | ones] stationary operand so the
softmax denominators accumulate in the same PSUM tile as Y^T. The
gathered K/V stay fp8 all the way into the matmuls (stationary fp8 x
moving bf16 is legal on the PE).

LayerNorm rstd is computed as exp(-0.5*ln(var+eps)) so the scalar
engine only ever needs the natural_log_exp table set plus gelu -- two
table loads total, both off the critical path (a dummy gelu during the
proj phase prefetches the gelu set).
"""

import numpy as np
import ml_dtypes

import concourse.bass as bass
import concourse.bacc as bacc
import concourse.tile as tile
from concourse import mybir
from concourse.bass import ts, ds
from concourse.bass_utils import run_bass_kernel_spmd

f32 = mybir.dt.float32
bf16 = mybir.dt.bfloat16
fp8 = mybir.dt.float8e4
AF = mybir.ActivationFunctionType
OP = mybir.AluOpType

B, T, C, H = 2, 2048, 1024, 16
DH = C // H          # 64
F = 4 * C            # 4096
NCORES = 8
GROUP = 4            # cores per batch
TQ = T // GROUP      # 512 own rows per core
NT = T // 128        # 16 key tiles
CCH = C // 128       # 8 contraction chunks over C
PAIRS = H // 2       # 8 head pairs
FT = F // 128        # 32 hidden tiles
QT = TQ // 128       # 4 own-row tiles
VW = DH + 1          # 65: V columns per head incl. ones column
GROUPS = [[0, 1, 2, 3], [4, 5, 6, 7]]

_CACHED = {}


def _bcast(ap, parts=128):
    """DRAM AP for a 1-D tensor broadcast across `parts` partitions."""
    return bass.AP(tensor=ap.tensor, offset=ap.offset, ap=[[0, parts]] + list(ap.ap))


def _build_program(trivial_ln1, trivial_ln2, trivial_b):
    nc = bacc.Bacc("TRN2", target_bir_lowering=False, debug=False,
                   num_devices=NCORES)

    xq = nc.dram_tensor("xq", [TQ, C], f32, kind="ExternalInput")
    # pre-tiled weights: [128 (c within chunk), CCH, out-features]
    wq = nc.dram_tensor("wq", [128, CCH, C], bf16, kind="ExternalInput")
    wk = nc.dram_tensor("wk", [128, CCH, C], bf16, kind="ExternalInput")
    wv = nc.dram_tensor("wv", [128, CCH, C], bf16, kind="ExternalInput")
    bqv = nc.dram_tensor("bq", [128, PAIRS], f32, kind="ExternalInput")
    bkv = nc.dram_tensor("bk", [128, PAIRS], f32, kind="ExternalInput")
    bvv = nc.dram_tensor("bv", [C], f32, kind="ExternalInput")
    ln1w = nc.dram_tensor("ln1w", [C], f32, kind="ExternalInput")
    ln1b = nc.dram_tensor("ln1b", [C], f32, kind="ExternalInput")
    ln2w = nc.dram_tensor("ln2w", [C], f32, kind="ExternalInput")
    ln2b = nc.dram_tensor("ln2b", [C], f32, kind="ExternalInput")
    wp = nc.dram_tensor("wp", [128, CCH, C], bf16, kind="ExternalInput")
    bp = nc.dram_tensor("bp", [C], f32, kind="ExternalInput")
    # wf pre-tiled per f'-tile: [FT, 128 (c), CCH, 128 (f')]
    wf = nc.dram_tensor("wf", [FT, 128, CCH, 128], bf16, kind="ExternalInput")
    bf_ = nc.dram_tensor("bf", [128, FT], f32, kind="ExternalInput")
    wm = nc.dram_tensor("wm", [F, C], bf16, kind="ExternalInput")
    bm = nc.dram_tensor("bm", [C], f32, kind="ExternalInput")
    out = nc.dram_tensor("out", [TQ, C], f32, kind="ExternalOutput")

    # collective scratch (Internal DRAM); chunk a covers head pairs 2a,2a+1
    ko = [nc.dram_tensor(f"ko{a}", [256, TQ], fp8, kind="Internal")
          for a in range(4)]
    kg = [nc.dram_tensor(f"kg{a}", [GROUP, 256, TQ], fp8, kind="Internal")
          for a in range(4)]
    vo = [nc.dram_tensor(f"vo{a}", [TQ, 4 * VW], fp8, kind="Internal")
          for a in range(4)]
    vg = [nc.dram_tensor(f"vg{a}", [GROUP, TQ, 4 * VW], fp8, kind="Internal")
          for a in range(4)]
    din = nc.dram_tensor("din", [64], bf16, kind="Internal")
    dout = nc.dram_tensor("dout", [GROUP * 64], bf16, kind="Internal")

    with tile.TileContext(nc) as tc:
        _emit(nc, tc, trivial_ln1, trivial_ln2, trivial_b,
              xq, wq, wk, wv, bqv, bkv, bvv, ln1w, ln1b, ln2w, ln2b,
              wp, bp, wf, bf_, wm, bm, out,
              ko, kg, vo, vg, din, dout)
    nc.compile()
    return nc


def _emit(nc, tc, trivial_ln1, trivial_ln2, trivial_b,
          xq, wq, wk, wv, bqv, bkv, bvv, ln1w, ln1b, ln2w, ln2b,
          wp, bp, wf, bf_, wm, bm, out,
          ko, kg, vo, vg, din, dout):
    from contextlib import ExitStack

    with ExitStack() as st:
        persist = st.enter_context(tc.tile_pool(name="persist", bufs=1))
        stat = st.enter_context(tc.tile_pool(name="stat", bufs=4))
        stream = st.enter_context(tc.tile_pool(name="stream", bufs=4))

        eps_t = persist.tile([128, 1], f32)
        nc.vector.memset(eps_t, 1e-5)

        # ---- dummy warm-up collective (gpsimd-issued input DMA so the
        # sync queue never waits on collective completion semaphores) ----
        d_t = persist.tile([1, 64], bf16)
        nc.vector.memset(d_t, 1.0)
        nc.gpsimd.dma_start(out=din.ap().rearrange("(p c) -> p c", p=1), in_=d_t)
        nc.gpsimd.collective_compute(
            "AllGather", OP.bypass, replica_groups=GROUPS,
            ins=[din.ap().opt()], outs=[dout.ap().opt()])

        # ---------------- pools ----------------
        stA = st.enter_context(ExitStack())
        pA = stA.enter_context(tc.tile_pool(name="pA", bufs=1, side="left"))
        pR = st.enter_context(tc.tile_pool(name="pR", bufs=1, side="right"))

        # persistent activations
        hT = pA.tile([128, QT, CCH, 128], bf16)
        xqs = pR.tile([128, QT, C], f32)          # own x rows (LN1 + residual)
        qT = pR.tile([128, PAIRS, TQ], bf16)
        v_sb = pR.tile([128, NT, H * VW], fp8)    # [tok, tile, 16*(DH+1)]
        ynT = pR.tile([128, PAIRS, TQ], bf16)

        # x rows first in the DMA queues (everything hangs off LN1)
        for i in range(QT):
            nc.sync.dma_start(out=xqs[:, i], in_=xq.ap()[ts(i, 128), :])
        wk_sb = pA.tile([128, CCH, C], bf16)
        nc.sync.dma_start(out=wk_sb, in_=wk.ap())
        wq_sb = pA.tile([128, CCH, C], bf16)
        nc.sync.dma_start(out=wq_sb, in_=wq.ap())
        wv_sb = pA.tile([128, CCH, C], bf16)
        nc.sync.dma_start(out=wv_sb, in_=wv.ap())
        if not trivial_b:
            bq_sb = pA.tile([128, PAIRS], f32)
            nc.sync.dma_start(out=bq_sb, in_=bqv.ap())
            bk_sb = pA.tile([128, PAIRS], f32)
            nc.sync.dma_start(out=bk_sb, in_=bkv.ap())
            bv_bc = pA.tile([128, C], f32)
            nc.sync.dma_start(out=bv_bc, in_=_bcast(bvv.ap()))
        if not trivial_ln1:
            ln1w_bc = pA.tile([128, C], f32)
            nc.sync.dma_start(out=ln1w_bc, in_=_bcast(ln1w.ap()))
            ln1b_bc = pA.tile([128, C], f32)
            nc.sync.dma_start(out=ln1b_bc, in_=_bcast(ln1b.ap()))
        else:
            ln1w_bc = ln1b_bc = None

        def layer_norm(x_t, w_bc, b_bc, out_ap, trivial):
            """x_t [128, C] f32 -> out_ap [128, C] bf16 (normalized + affine).

            rstd = exp(-0.5 * ln(var + eps)) keeps ACT on the ln/exp table."""
            stats = stat.tile([128, 2, nc.vector.BN_STATS_DIM], f32, name="stats", bufs=6)
            nc.vector.bn_stats(out=stats[:, 0, :], in_=x_t[:, 0:512])
            nc.vector.bn_stats(out=stats[:, 1, :], in_=x_t[:, 512:1024])
            mv = stat.tile([128, nc.vector.BN_AGGR_DIM], f32, name="mv", bufs=6)
            nc.vector.bn_aggr(out=mv, in_=stats)
            lnv = stat.tile([128, 1], f32, name="lnv", bufs=6)
            nc.scalar.activation(lnv, mv[:, 1:2], AF.Ln, bias=eps_t)
            rstd = stat.tile([128, 1], f32, name="rstd", bufs=6)
            nc.scalar.activation(rstd, lnv, AF.Exp, scale=-0.5)
            if trivial:
                nc.vector.tensor_scalar(out=out_ap, in0=x_t, scalar1=mv[:, 0:1],
                                        scalar2=rstd, op0=OP.subtract, op1=OP.mult)
            else:
                t1 = stat.tile([128, C], f32, name="t1", tag="ln_t1")
                nc.vector.tensor_scalar(out=t1, in0=x_t, scalar1=mv[:, 0:1],
                                        scalar2=rstd, op0=OP.subtract, op1=OP.mult)
                nc.vector.tensor_mul(t1, t1, w_bc)
                nc.vector.tensor_add(out_ap, t1, b_bc)

        # ---- LN1 over own 4 tiles ----
        for i in range(QT):
            h_t = stream.tile([128, C], bf16, name="h_t", tag="h_t", bufs=6)
            layer_norm(xqs[:, i], ln1w_bc, ln1b_bc, h_t, trivial_ln1)
            nc.sync.dma_start_transpose(hT[:, i], h_t[:])

        # ---- own K/Q/V per 2-pair chunk, AllGathers fired ASAP ----
        kvst = st.enter_context(ExitStack())
        kv_ps = kvst.enter_context(tc.tile_pool(name="kv_ps", bufs=2, space="PSUM"))
        vos_pool = kvst.enter_context(tc.tile_pool(name="vos", bufs=2))

        def chunk(a, on_act):
            # K for pairs 2a, 2a+1  -> AG;  Q same pairs;  V heads 4a..4a+4 -> AG
            for e in range(2):
                j = 2 * a + e
                ps = kv_ps.tile([128, TQ], f32, name="ps_k", tag="ps_kv")
                for c in range(CCH):
                    nc.tensor.matmul(ps, wk_sb[:, c, ts(j, 128)],
                                     hT[:, 0:QT, c, :],
                                     start=(c == 0), stop=(c == CCH - 1))
                kt_t = stream.tile([128, TQ], fp8, name="kt_t", tag="kt_t", bufs=4)
                if trivial_b:
                    if on_act:
                        nc.scalar.activation(kt_t, ps, AF.Identity)
                    else:
                        nc.vector.tensor_copy(kt_t, ps)
                else:
                    nc.vector.tensor_scalar(out=kt_t, in0=ps,
                                            scalar1=bk_sb[:, j:j + 1],
                                            scalar2=None, op0=OP.add)
                nc.sync.dma_start(out=ko[a].ap()[ts(e, 128), :], in_=kt_t)
            nc.gpsimd.collective_compute(
                "AllGather", OP.bypass, replica_groups=GROUPS,
                ins=[ko[a].ap().opt()], outs=[kg[a].ap().opt()])
            for e in range(2):
                j = 2 * a + e
                ps = kv_ps.tile([128, TQ], f32, name="ps_q", tag="ps_kv")
                for c in range(CCH):
                    nc.tensor.matmul(ps, wq_sb[:, c, ts(j, 128)],
                                     hT[:, 0:QT, c, :],
                                     start=(c == 0), stop=(c == CCH - 1))
                if trivial_b:
                    if on_act:
                        nc.scalar.activation(qT[:, j], ps, AF.Identity)
                    else:
                        nc.vector.tensor_copy(qT[:, j], ps)
                else:
                    nc.vector.tensor_scalar(out=qT[:, j], in0=ps,
                                            scalar1=bq_sb[:, j:j + 1],
                                            scalar2=None, op0=OP.add)
            vos = vos_pool.tile([128, QT, 4 * VW], fp8, name="vos", tag="vos")
            vosv = vos.rearrange("p q (h x) -> p q h x", x=VW)
            nc.vector.memset(vosv[:, :, :, DH:DH + 1], 1.0)
            for i in range(QT):
                ps = kv_ps.tile([128, TQ], f32, name="ps_v", tag="ps_kv")
                for c in range(CCH):
                    nc.tensor.matmul(ps[:, 0:256], hT[:, i, c, :],
                                     wv_sb[:, c, ds(256 * a, 256)],
                                     start=(c == 0), stop=(c == CCH - 1))
                dst = vosv[:, i, :, 0:DH]
                psv = ps[:, 0:256].rearrange("p (h x) -> p h x", x=DH)
                if trivial_b:
                    if on_act:
                        nc.scalar.activation(dst, psv, AF.Identity)
                    else:
                        nc.vector.tensor_copy(dst, psv)
                else:
                    bvw = bv_bc[:, ds(256 * a, 256)].rearrange(
                        "p (h x) -> p h x", x=DH)
                    nc.vector.tensor_add(dst, psv, bvw)
            nc.sync.dma_start(
                out=vo[a].ap().rearrange("(q p) f -> p q f", p=128), in_=vos)
            nc.gpsimd.collective_compute(
                "AllGather", OP.bypass, replica_groups=GROUPS,
                ins=[vo[a].ap().opt()], outs=[vg[a].ap().opt()])

        def v_in(a):
            # gathered V chunk -> resident v_sb, per key tile
            for t in range(NT):
                r, w = t // 4, t % 4
                nc.sync.dma_start(
                    out=v_sb[:, t, ds(4 * VW * a, 4 * VW)],
                    in_=vg[a].ap()[r, ts(w, 128), :])

        # ---- attention (kT/V from the AllGathers, fp8 stationaries) ----
        scale = 1.0 / float(np.sqrt(DH))
        att_ctx = st.enter_context(ExitStack())
        s_ps = att_ctx.enter_context(tc.tile_pool(name="s_ps", bufs=2, space="PSUM"))
        y_ps = att_ctx.enter_context(tc.tile_pool(name="y_ps", bufs=1, space="PSUM"))
        kq_sb = att_ctx.enter_context(tc.tile_pool(name="kq_sb", bufs=2))
        att_sb = att_ctx.enter_context(tc.tile_pool(name="att_sb", bufs=3))
        resc = {}

        def attention_main(j):
            a, e = j // 2, j % 2
            kT_j = kq_sb.tile([128, T], fp8, name="kT_j", tag="kT_j")
            for r in range(GROUP):
                nc.sync.dma_start(out=kT_j[:, ds(TQ * r, TQ)],
                                  in_=kg[a].ap()[r, ts(e, 128), :])
            ps_y1 = y_ps.tile([VW, 512], f32, name="ps_y1", tag="ps_y1")
            ps_y2 = y_ps.tile([VW, 512], f32, name="ps_y2", tag="ps_y2")
            for cidx in range(NT):
                ps_s = s_ps.tile([128, 1024], f32, name="ps_s", tag="ps_s")
                nc.tensor.matmul(ps_s[:, 0:512],
                                 kT_j[0:64, ts(cidx, 128)],
                                 qT[0:64, j, :], start=True, stop=True)
                nc.tensor.matmul(ps_s[:, 512:1024],
                                 kT_j[64:128, ts(cidx, 128)],
                                 qT[64:128, j, :], start=True, stop=True,
                                 tile_position=(64, 0))
                pT = att_sb.tile([128, 1024], bf16, name="pT", tag="pT")
                nc.scalar.activation(pT, ps_s, AF.Exp, scale=scale)
                nc.tensor.matmul(ps_y1,
                                 v_sb[:, cidx, ds(VW * 2 * j, VW)],
                                 pT[:, 0:512],
                                 start=(cidx == 0), stop=(cidx == NT - 1))
                nc.tensor.matmul(ps_y2,
                                 v_sb[:, cidx, ds(VW * (2 * j + 1), VW)],
                                 pT[:, 512:1024],
                                 start=(cidx == 0), stop=(cidx == NT - 1))
            # copy Y and the sums row out of PSUM right away so the
            # accumulator banks free up for the next pair; the sums
            # staging copy also moves them to SBUF partition 0
            # (custom-DVE ops mis-read PSUM at a partition offset)
            for u, ps_y in ((0, ps_y1), (1, ps_y2)):
                ycp = att_sb.tile([64, 512], f32, name="ycp", tag=f"ycp{u}", bufs=2)
                nc.vector.tensor_copy(ycp, ps_y[0:DH, :])
                rs0 = att_sb.tile([1, 512], f32, name="rs0", tag=f"rs0{u}", bufs=2)
                nc.vector.tensor_copy(rs0, ps_y[DH:DH + 1, :])
                rs = att_sb.tile([1, 512], f32, name="rs", tag=f"rs{u}", bufs=2)
                nc.vector.reciprocal_approx_fast(rs, rs0)
                resc[(j, u)] = (ycp, rs)

        def attention_rescale(j):
            # partition_broadcast lives on gpsimd: emitted one chunk late so
            # every AllGather trigger is already queued ahead of it
            for u in range(2):
                ycp, rs = resc.pop((j, u))
                bc = att_sb.tile([64, 512], f32, name="bc", tag="bc")
                nc.gpsimd.partition_broadcast(bc, rs)
                nc.vector.tensor_mul(ynT[64 * u:64 * u + 64, j, :], ycp, bc)

        chunk(0, True)
        v_in(0)
        attention_main(0)
        chunk(1, False)
        v_in(1)
        attention_rescale(0)
        attention_main(1)
        chunk(2, False)
        v_in(2)
        attention_rescale(1)
        attention_main(2)
        chunk(3, False)
        v_in(3)
        attention_rescale(2)
        attention_main(3)
        attention_rescale(3)
        for j in range(4, PAIRS):
            attention_main(j)
            attention_rescale(j)
        att_ctx.close()
        kvst.close()
        stA.close()

        # wp + proj-era constants (DMAs overlap the attention phase)
        wp_sb = pR.tile([128, CCH, C], bf16)
        nc.sync.dma_start(out=wp_sb, in_=wp.ap())
        if not trivial_ln2:
            ln2w_bc = pR.tile([128, C], f32)
            nc.sync.dma_start(out=ln2w_bc, in_=_bcast(ln2w.ap()))
            ln2b_bc = pR.tile([128, C], f32)
            nc.sync.dma_start(out=ln2b_bc, in_=_bcast(ln2b.ap()))
        else:
            ln2w_bc = ln2b_bc = None
        if not trivial_b:
            bp_bc = pR.tile([128, C], f32)
            nc.sync.dma_start(out=bp_bc, in_=_bcast(bp.ap()))

        # ---- attn projection + residual + LN2 + h2^T ----
        pD = st.enter_context(tc.tile_pool(name="pD", bufs=1, side="left"))
        x2 = pD.tile([128, QT, C], f32)
        h2T = pD.tile([128, QT, CCH, 128], bf16)
        bfc_sb = pD.tile([128, FT], f32)
        nc.sync.dma_start(out=bfc_sb, in_=bf_.ap())
        bm_bc = pD.tile([128, C], f32)
        nc.sync.dma_start(out=bm_bc, in_=_bcast(bm.ap()))

        with tc.tile_pool(name="ap_ps", bufs=2, space="PSUM") as ap_ps:
            for i in range(QT):
                if not trivial_b:
                    nc.vector.tensor_add(xqs[:, i], xqs[:, i], bp_bc)
                for n in range(C // 512):
                    ps = ap_ps.tile([128, 512], f32, name="ps_a", tag="ps_a")
                    for j in range(PAIRS):
                        nc.tensor.matmul(ps, ynT[:, j, ts(i, 128)],
                                         wp_sb[:, j, ds(512 * n, 512)],
                                         start=(j == 0), stop=(j == PAIRS - 1))
                    nc.vector.tensor_add(x2[:, i, ds(512 * n, 512)], ps,
                                         xqs[:, i, ds(512 * n, 512)])
                h2_t = stream.tile([128, C], bf16, name="h2_t", tag="h_t", bufs=6)
                layer_norm(x2[:, i, :], ln2w_bc, ln2b_bc, h2_t, trivial_ln2)
                nc.sync.dma_start_transpose(h2T[:, i], h2_t[:])
                if i == QT - 1:
                    # prefetch the gelu table while proj/LN2 finish
                    gd = stat.tile([128, 1], f32, name="gd", bufs=6)
                    nc.scalar.activation(gd, eps_t, AF.Gelu_apprx_tanh)

        # ---- MLP ----
        # fold the mlp_proj bias into the residual copy while fc runs
        for i in range(QT):
            nc.vector.tensor_add(x2[:, i, :], x2[:, i, :], bm_bc)
        gT = pD.tile([128, FT, TQ], bf16)
        with tc.tile_pool(name="fc_ps", bufs=4, space="PSUM") as fc_ps, \
             tc.tile_pool(name="wf_sb", bufs=6) as wf_pool:
            for t in range(FT):
                wf_t = wf_pool.tile([128, CCH, 128], bf16, name="wf_t", tag="wf_t")
                nc.sync.dma_start(out=wf_t, in_=wf.ap()[t])
                ps = fc_ps.tile([128, 512], f32, name="ps_f", tag="ps_f")
                for c in range(CCH):
                    nc.tensor.matmul(ps, wf_t[:, c, :], h2T[:, 0:QT, c, :],
                                     start=(c == 0), stop=(c == CCH - 1))
                nc.scalar.activation(gT[:, t, :], ps, AF.Gelu_apprx_tanh,
                                     bias=bfc_sb[:, t:t + 1], scale=1.0)

        with tc.tile_pool(name="m_ps", bufs=1, space="PSUM") as m_ps, \
             tc.tile_pool(name="wm_sb", bufs=6) as wm_pool, \
             tc.tile_pool(name="out_sb", bufs=2) as out_pool:
            ps_m = [m_ps.tile([128, 512], f32, name=f"ps_m{k}", tag=f"ps_m{k}")
                    for k in range(8)]
            for t in range(FT):
                wm_t = wm_pool.tile([128, C], bf16, name="wm_t", tag="wm_t")
                nc.sync.dma_start(out=wm_t, in_=wm.ap()[ts(t, 128), :])
                for i in range(QT):
                    for n in range(C // 512):
                        nc.tensor.matmul(ps_m[i * 2 + n], gT[:, t, ts(i, 128)],
                                         wm_t[:, ds(512 * n, 512)],
                                         start=(t == 0), stop=(t == FT - 1))
            for i in range(QT):
                out_t = out_pool.tile([128, C], f32, name="out_t", tag="out_t")
                for n in range(C // 512):
                    # DMA per 512-col chunk right away so the tail drains fast
                    nc.vector.tensor_add(out_t[:, ds(512 * n, 512)],
                                         ps_m[i * 2 + n],
                                         x2[:, i, ds(512 * n, 512)])
                    nc.sync.dma_start(out=out.ap()[ts(i, 128), ds(512 * n, 512)],
                                      in_=out_t[:, ds(512 * n, 512)])


def _get_program(trivial_ln1, trivial_ln2, trivial_b):
    key = (trivial_ln1, trivial_ln2, trivial_b)
    if key not in _CACHED:
        _CACHED[key] = _build_program(trivial_ln1, trivial_ln2, trivial_b)
    return _CACHED[key]


def _tile_proj_weight(w):
    # [C, N] f32 -> [128, CCH, N] bf16 with partition = c % 128, chunk = c // 128
    w = np.asarray(w, np.float32).reshape(CCH, 128, -1)
    return np.ascontiguousarray(w.transpose(1, 0, 2).astype(ml_dtypes.bfloat16))


def _prep_in_maps(inputs):
    fl = lambda a: np.ascontiguousarray(np.asarray(a, np.float32))
    x = fl(inputs["x"])
    attn_w = fl(inputs["attn_w"])
    attn_b = fl(inputs["attn_b"])
    wf_full = fl(inputs["fc_w"])  # [C, F]
    # wf tiled: [FT, 128(c), CCH, 128(f')]
    wf_t = wf_full.reshape(CCH, 128, FT, 128).transpose(2, 1, 0, 3)
    wf_t = np.ascontiguousarray(wf_t.astype(ml_dtypes.bfloat16))
    pb = lambda b: np.ascontiguousarray(
        np.asarray(b, np.float32).reshape(-1, 128).T)  # [128, tiles]
    shared = {
        "wq": _tile_proj_weight(attn_w[:, 0:C]),
        "wk": _tile_proj_weight(attn_w[:, C:2 * C]),
        "wv": _tile_proj_weight(attn_w[:, 2 * C:3 * C]),
        "bq": pb(attn_b[0:C]), "bk": pb(attn_b[C:2 * C]),
        "bv": fl(attn_b[2 * C:3 * C]),
        "ln1w": fl(inputs["ln1_w"]), "ln1b": fl(inputs["ln1_b"]),
        "ln2w": fl(inputs["ln2_w"]), "ln2b": fl(inputs["ln2_b"]),
        "wp": _tile_proj_weight(inputs["attn_proj_w"]),
        "bp": fl(inputs["attn_proj_b"]),
        "wf": wf_t, "bf": pb(inputs["fc_b"]),
        "wm": np.ascontiguousarray(fl(inputs["mlp_proj_w"]).astype(ml_dtypes.bfloat16)),
        "bm": fl(inputs["mlp_proj_b"]),
    }
    in_maps = []
    for core in range(NCORES):
        b, r = core // GROUP, core % GROUP
        in_maps.append({
            "xq": np.ascontiguousarray(x[b, TQ * r:TQ * (r + 1)]),
            **shared,
        })
    return in_maps


def run(inputs, trace=False):
    trivial_ln1 = bool(np.all(np.asarray(inputs["ln1_w"]) == 1.0)
                       and np.all(np.asarray(inputs["ln1_b"]) == 0.0))
    trivial_ln2 = bool(np.all(np.asarray(inputs["ln2_w"]) == 1.0)
                       and np.all(np.asarray(inputs["ln2_b"]) == 0.0))
    trivial_b = bool(np.all(np.asarray(inputs["attn_b"]) == 0.0))
    nc = _get_program(trivial_ln1, trivial_ln2, trivial_b)
    in_maps = _prep_in_maps(inputs)
    res = run_bass_kernel_spmd(nc, in_maps, core_ids=list(range(NCORES)),
                               trace=trace)
    out = np.empty((B, T, C), np.float32)
    for core in range(NCORES):
        b, r = core // GROUP, core % GROUP
        out[b, TQ * r:TQ * (r + 1)] = res.results[core]["out"]
    return out, res


def kernel(**inputs):
    out, _ = run(inputs, trace=False)
    return out


# revision 10
# speedup vs baseline: 1.0440x; 1.0311x over previous
"""Trainium2 Bass kernel for a GPT-2 style transformer block (pre-LN, no mask).

Reference shapes: x [B=2, T=2048, C=1024], H=16 heads, MLP hidden 4C=4096.

Sharding (8 NeuronCores): data-parallel over B (cores 0-3 -> batch 0,
cores 4-7 -> batch 1); within each 4-core group the 2048 rows are split
512 per core. Each core computes LN1 + Q/K/V only for its OWN 512 rows,
then K^T and V are AllGather'd across the 4-core group in fp8e4 (2-pair
chunks, K and V alternating) so attention/proj/MLP stay fully local per
core. A tiny dummy AllGather issued at t=0 absorbs the first-collective
latency (ncfw warmup / core-start skew); its input DMA is issued from
the gpsimd queue so it cannot serialize the sync-engine DMA stream.

Compute layout: activations feeding matmul contractions are kept
feature-major ("transposed", [C, t]) via the DMA xbar transpose; scores
are computed as S^T = K Q^T per head ([tk, tq]) with two heads packed
into the 128-wide contraction via row tiling (the two matmuls run
concurrently in disjoint PE row groups); exp runs on the scalar engine
straight out of PSUM; P @ V uses a [V | ones] stationary operand so the
softmax denominators accumulate in the same PSUM tile as Y^T. The
gathered K/V stay fp8 all the way into the matmuls (stationary fp8 x
moving bf16 is legal on the PE).

LayerNorm rstd is computed as exp(-0.5*ln(var+eps)) so the scalar
engine only ever needs the natural_log_exp table set plus gelu -- two
table loads total, both off the critical path (a dummy gelu during the
proj phase prefetches the gelu set).
"""

import numpy as np
import ml_dtypes

import concourse.bass as bass
import concourse.bacc as bacc
import concourse.tile as tile
from concourse import mybir
from concourse.bass import ts, ds
from concourse.bass_utils import run_bass_kernel_spmd

f32 = mybir.dt.float32
bf16 = mybir.dt.bfloat16
fp8 = mybir.dt.float8e4
AF = mybir.ActivationFunctionType
OP = mybir.AluOpType

B, T, C, H = 2, 2048, 1024, 16
DH = C // H          # 64
F = 4 * C            # 4096
NCORES = 8
GROUP = 4            # cores per batch
TQ = T // GROUP      # 512 own rows per core
NT = T // 128        # 16 key tiles
CCH = C // 128       # 8 contraction chunks over C
PAIRS = H // 2       # 8 head pairs
FT = F // 128        # 32 hidden tiles
QT = TQ // 128       # 4 own-row tiles
VW = DH + 1          # 65: V columns per head incl. ones column
GROUPS = [[0, 1, 2, 3], [4, 5, 6, 7]]

_CACHED = {}


def _bcast(ap, parts=128):
    """DRAM AP for a 1-D tensor broadcast across `parts` partitions."""
    return bass.AP(tensor=ap.tensor, offset=ap.offset, ap=[[0, parts]] + list(ap.ap))


def _build_program(trivial_ln1, trivial_ln2, trivial_b):
    nc = bacc.Bacc("TRN2", target_bir_lowering=False, debug=False,
                   num_devices=NCORES)

    xq = nc.dram_tensor("xq", [TQ, C], f32, kind="ExternalInput")
    # pre-tiled weights: [128 (c within chunk), CCH, out-features]
    wq = nc.dram_tensor("wq", [128, CCH, C], bf16, kind="ExternalInput")
    wk = nc.dram_tensor("wk", [128, CCH, C], bf16, kind="ExternalInput")
    wv = nc.dram_tensor("wv", [128, CCH, C], bf16, kind="ExternalInput")
    bqv = nc.dram_tensor("bq", [128, PAIRS], f32, kind="ExternalInput")
    bkv = nc.dram_tensor("bk", [128, PAIRS], f32, kind="ExternalInput")
    bvv = nc.dram_tensor("bv", [C], f32, kind="ExternalInput")
    ln1w = nc.dram_tensor("ln1w", [C], f32, kind="ExternalInput")
    ln1b = nc.dram_tensor("ln1b", [C], f32, kind="ExternalInput")
    ln2w = nc.dram_tensor("ln2w", [C], f32, kind="ExternalInput")
    ln2b = nc.dram_tensor("ln2b", [C], f32, kind="ExternalInput")
    wp = nc.dram_tensor("wp", [128, CCH, C], bf16, kind="ExternalInput")
    bp = nc.dram_tensor("bp", [C], f32, kind="ExternalInput")
    # wf pre-tiled per f'-tile: [FT, 128 (c), CCH, 128 (f')]
    wf = nc.dram_tensor("wf", [FT, 128, CCH, 128], bf16, kind="ExternalInput")
    bf_ = nc.dram_tensor("bf", [128, FT], f32, kind="ExternalInput")
    wm = nc.dram_tensor("wm", [F, C], bf16, kind="ExternalInput")
    bm = nc.dram_tensor("bm", [C], f32, kind="ExternalInput")
    idw = nc.dram_tensor("idw", [128, 128], f32, kind="ExternalInput")
    out = nc.dram_tensor("out", [TQ, C], f32, kind="ExternalOutput")

    # collective scratch (Internal DRAM); chunk a covers head pairs 2a,2a+1
    ko = [nc.dram_tensor(f"ko{a}", [256, TQ], fp8, kind="Internal")
          for a in range(4)]
    kg = [nc.dram_tensor(f"kg{a}", [GROUP, 256, TQ], fp8, kind="Internal")
          for a in range(4)]
    vo = [nc.dram_tensor(f"vo{a}", [TQ, 4 * VW], fp8, kind="Internal")
          for a in range(4)]
    vg = [nc.dram_tensor(f"vg{a}", [GROUP, TQ, 4 * VW], fp8, kind="Internal")
          for a in range(4)]
    din = nc.dram_tensor("din", [64], bf16, kind="Internal")
    dout = nc.dram_tensor("dout", [GROUP * 64], bf16, kind="Internal")

    with tile.TileContext(nc) as tc:
        _emit(nc, tc, trivial_ln1, trivial_ln2, trivial_b,
              xq, wq, wk, wv, bqv, bkv, bvv, ln1w, ln1b, ln2w, ln2b,
              wp, bp, wf, bf_, wm, bm, idw, out,
              ko, kg, vo, vg, din, dout)
    nc.compile()
    return nc


def _emit(nc, tc, trivial_ln1, trivial_ln2, trivial_b,
          xq, wq, wk, wv, bqv, bkv, bvv, ln1w, ln1b, ln2w, ln2b,
          wp, bp, wf, bf_, wm, bm, idw, out,
          ko, kg, vo, vg, din, dout):
    from contextlib import ExitStack

    with ExitStack() as st:
        persist = st.enter_context(tc.tile_pool(name="persist", bufs=1))
        stat = st.enter_context(tc.tile_pool(name="stat", bufs=4))
        stream = st.enter_context(tc.tile_pool(name="stream", bufs=4))

        eps_t = persist.tile([128, 1], f32)
        nc.vector.memset(eps_t, 1e-5)

        # ---- dummy warm-up collective: no input DMA at all (contents are
        # irrelevant) so the trigger fires the moment the preamble ends and
        # the ~40us ncfw first-collective warmup overlaps LN1/QKV ----
        nc.gpsimd.collective_compute(
            "AllGather", OP.bypass, replica_groups=GROUPS,
            ins=[din.ap().opt()], outs=[dout.ap().opt()])

        # ---------------- pools ----------------
        stA = st.enter_context(ExitStack())
        pA = stA.enter_context(tc.tile_pool(name="pA", bufs=1, side="left"))
        pR = st.enter_context(tc.tile_pool(name="pR", bufs=1, side="right"))

        # persistent activations
        hT = pA.tile([128, QT, CCH, 128], bf16)
        xqs = pR.tile([128, QT, C], f32)          # own x rows (LN1 + residual)
        qT = pR.tile([128, PAIRS, TQ], bf16)
        v_sb = pR.tile([128, NT, H * VW], fp8)    # [tok, tile, 16*(DH+1)]
        ynT = pR.tile([128, PAIRS, TQ], bf16)

        # x rows first in the DMA queues (everything hangs off LN1)
        for i in range(QT):
            nc.sync.dma_start(out=xqs[:, i], in_=xq.ap()[ts(i, 128), :])
        wk_sb = pA.tile([128, CCH, C], bf16)
        nc.sync.dma_start(out=wk_sb, in_=wk.ap())
        wq_sb = pA.tile([128, CCH, C], bf16)
        nc.sync.dma_start(out=wq_sb, in_=wq.ap())
        wv_sb = pA.tile([128, CCH, C], bf16)
        nc.sync.dma_start(out=wv_sb, in_=wv.ap())
        if not trivial_b:
            bq_sb = pA.tile([128, PAIRS], f32)
            nc.sync.dma_start(out=bq_sb, in_=bqv.ap())
            bk_sb = pA.tile([128, PAIRS], f32)
            nc.sync.dma_start(out=bk_sb, in_=bkv.ap())
            bv_bc = pA.tile([128, C], f32)
            nc.sync.dma_start(out=bv_bc, in_=_bcast(bvv.ap()))
        if not trivial_ln1:
            ln1w_bc = pA.tile([128, C], f32)
            nc.sync.dma_start(out=ln1w_bc, in_=_bcast(ln1w.ap()))
            ln1b_bc = pA.tile([128, C], f32)
            nc.sync.dma_start(out=ln1b_bc, in_=_bcast(ln1b.ap()))
        else:
            ln1w_bc = ln1b_bc = None

        def layer_norm(x_t, w_bc, b_bc, out_ap, trivial):
            """x_t [128, C] f32 -> out_ap [128, C] bf16 (normalized + affine).

            rstd = exp(-0.5 * ln(var + eps)) keeps ACT on the ln/exp table."""
            stats = stat.tile([128, 2, nc.vector.BN_STATS_DIM], f32, name="stats", bufs=6)
            nc.vector.bn_stats(out=stats[:, 0, :], in_=x_t[:, 0:512])
            nc.vector.bn_stats(out=stats[:, 1, :], in_=x_t[:, 512:1024])
            mv = stat.tile([128, nc.vector.BN_AGGR_DIM], f32, name="mv", bufs=6)
            nc.vector.bn_aggr(out=mv, in_=stats)
            rstd = stat.tile([128, 1], f32, name="rstd", bufs=6)
            nc.scalar.activation(rstd, mv[:, 1:2], AF.Sqrt, bias=eps_t)
            nc.vector.reciprocal(rstd, rstd)
            if trivial:
                nc.vector.tensor_scalar(out=out_ap, in0=x_t, scalar1=mv[:, 0:1],
                                        scalar2=rstd, op0=OP.subtract, op1=OP.mult)
            else:
                t1 = stat.tile([128, C], f32, name="t1", tag="ln_t1")
                nc.vector.tensor_scalar(out=t1, in0=x_t, scalar1=mv[:, 0:1],
                                        scalar2=rstd, op0=OP.subtract, op1=OP.mult)
                nc.vector.tensor_mul(t1, t1, w_bc)
                nc.vector.tensor_add(out_ap, t1, b_bc)

        # ---- LN1 over own 4 tiles ----
        for i in range(QT):
            h_t = stream.tile([128, C], bf16, name="h_t", tag="h_t", bufs=6)
            layer_norm(xqs[:, i], ln1w_bc, ln1b_bc, h_t, trivial_ln1)
            nc.sync.dma_start_transpose(hT[:, i], h_t[:])

        # ---- own K/Q/V per 2-pair chunk, AllGathers fired ASAP ----
        kvst = st.enter_context(ExitStack())
        kv_ps = kvst.enter_context(tc.tile_pool(name="kv_ps", bufs=2, space="PSUM"))
        vos_pool = kvst.enter_context(tc.tile_pool(name="vos", bufs=2))

        def chunk(a, on_act):
            # K for pairs 2a, 2a+1  -> AG;  Q same pairs;  V heads 4a..4a+4 -> AG
            for e in range(2):
                j = 2 * a + e
                ps = kv_ps.tile([128, TQ], f32, name="ps_k", tag="ps_kv")
                for c in range(CCH):
                    nc.tensor.matmul(ps, wk_sb[:, c, ts(j, 128)],
                                     hT[:, 0:QT, c, :],
                                     start=(c == 0), stop=(c == CCH - 1))
                kt_t = stream.tile([128, TQ], fp8, name="kt_t", tag="kt_t", bufs=4)
                if trivial_b:
                    if on_act:
                        nc.scalar.activation(kt_t, ps, AF.Identity)
                    else:
                        nc.vector.tensor_copy(kt_t, ps)
                else:
                    nc.vector.tensor_scalar(out=kt_t, in0=ps,
                                            scalar1=bk_sb[:, j:j + 1],
                                            scalar2=None, op0=OP.add)
                nc.sync.dma_start(out=ko[a].ap()[ts(e, 128), :], in_=kt_t)
            nc.gpsimd.collective_compute(
                "AllGather", OP.bypass, replica_groups=GROUPS,
                ins=[ko[a].ap().opt()], outs=[kg[a].ap().opt()])
            for e in range(2):
                j = 2 * a + e
                ps = kv_ps.tile([128, TQ], f32, name="ps_q", tag="ps_kv")
                for c in range(CCH):
                    nc.tensor.matmul(ps, wq_sb[:, c, ts(j, 128)],
                                     hT[:, 0:QT, c, :],
                                     start=(c == 0), stop=(c == CCH - 1))
                if trivial_b:
                    if on_act:
                        nc.scalar.activation(qT[:, j], ps, AF.Identity)
                    else:
                        nc.vector.tensor_copy(qT[:, j], ps)
                else:
                    nc.vector.tensor_scalar(out=qT[:, j], in0=ps,
                                            scalar1=bq_sb[:, j:j + 1],
                                            scalar2=None, op0=OP.add)
            vos = vos_pool.tile([128, QT, 4 * VW], fp8, name="vos", tag="vos")
            vosv = vos.rearrange("p q (h x) -> p q h x", x=VW)
            nc.vector.memset(vosv[:, :, :, DH:DH + 1], 1.0)
            for i in range(QT):
                ps = kv_ps.tile([128, TQ], f32, name="ps_v", tag="ps_kv")
                for c in range(CCH):
                    nc.tensor.matmul(ps[:, 0:256], hT[:, i, c, :],
                                     wv_sb[:, c, ds(256 * a, 256)],
                                     start=(c == 0), stop=(c == CCH - 1))
                dst = vosv[:, i, :, 0:DH]
                psv = ps[:, 0:256].rearrange("p (h x) -> p h x", x=DH)
                if trivial_b:
                    if on_act:
                        nc.scalar.activation(dst, psv, AF.Identity)
                    else:
                        nc.vector.tensor_copy(dst, psv)
                else:
                    bvw = bv_bc[:, ds(256 * a, 256)].rearrange(
                        "p (h x) -> p h x", x=DH)
                    nc.vector.tensor_add(dst, psv, bvw)
            nc.sync.dma_start(
                out=vo[a].ap().rearrange("(q p) f -> p q f", p=128), in_=vos)
            nc.gpsimd.collective_compute(
                "AllGather", OP.bypass, replica_groups=GROUPS,
                ins=[vo[a].ap().opt()], outs=[vg[a].ap().opt()])

        def v_in(a):
            # gathered V chunk -> resident v_sb, per key tile
            for t in range(NT):
                r, w = t // 4, t % 4
                nc.sync.dma_start(
                    out=v_sb[:, t, ds(4 * VW * a, 4 * VW)],
                    in_=vg[a].ap()[r, ts(w, 128), :])

        # ---- attention (kT/V from the AllGathers, fp8 stationaries) ----
        scale = 1.0 / float(np.sqrt(DH))
        att_ctx = st.enter_context(ExitStack())
        s_ps = att_ctx.enter_context(tc.tile_pool(name="s_ps", bufs=2, space="PSUM"))
        y_ps = att_ctx.enter_context(tc.tile_pool(name="y_ps", bufs=1, space="PSUM"))
        kq_sb = att_ctx.enter_context(tc.tile_pool(name="kq_sb", bufs=2))
        att_sb = att_ctx.enter_context(tc.tile_pool(name="att_sb", bufs=3))
        resc = {}

        def attention_main(j):
            a, e = j // 2, j % 2
            kT_j = kq_sb.tile([128, T], fp8, name="kT_j", tag="kT_j")
            for r in range(GROUP):
                nc.sync.dma_start(out=kT_j[:, ds(TQ * r, TQ)],
                                  in_=kg[a].ap()[r, ts(e, 128), :])
            ps_y1 = y_ps.tile([VW, 512], f32, name="ps_y1", tag="ps_y1")
            ps_y2 = y_ps.tile([VW, 512], f32, name="ps_y2", tag="ps_y2")
            for cidx in range(NT):
                ps_s = s_ps.tile([128, 1024], f32, name="ps_s", tag="ps_s")
                nc.tensor.matmul(ps_s[:, 0:512],
                                 kT_j[0:64, ts(cidx, 128)],
                                 qT[0:64, j, :], start=True, stop=True)
                nc.tensor.matmul(ps_s[:, 512:1024],
                                 kT_j[64:128, ts(cidx, 128)],
                                 qT[64:128, j, :], start=True, stop=True,
                                 tile_position=(64, 0))
                pT = att_sb.tile([128, 1024], bf16, name="pT", tag="pT")
                nc.scalar.activation(pT, ps_s, AF.Exp, scale=scale)
                nc.tensor.matmul(ps_y1,
                                 v_sb[:, cidx, ds(VW * 2 * j, VW)],
                                 pT[:, 0:512],
                                 start=(cidx == 0), stop=(cidx == NT - 1))
                nc.tensor.matmul(ps_y2,
                                 v_sb[:, cidx, ds(VW * (2 * j + 1), VW)],
                                 pT[:, 512:1024],
                                 start=(cidx == 0), stop=(cidx == NT - 1))
            # copy Y and the sums row out of PSUM right away so the
            # accumulator banks free up for the next pair; the sums
            # staging copy also moves them to SBUF partition 0
            # (custom-DVE ops mis-read PSUM at a partition offset)
            for u, ps_y in ((0, ps_y1), (1, ps_y2)):
                ycp = att_sb.tile([64, 512], f32, name="ycp", tag=f"ycp{u}", bufs=2)
                nc.vector.tensor_copy(ycp, ps_y[0:DH, :])
                rs0 = att_sb.tile([1, 512], f32, name="rs0", tag=f"rs0{u}", bufs=2)
                nc.vector.tensor_copy(rs0, ps_y[DH:DH + 1, :])
                rs = att_sb.tile([1, 512], f32, name="rs", tag=f"rs{u}", bufs=2)
                nc.vector.reciprocal_approx_fast(rs, rs0)
                resc[(j, u)] = (ycp, rs)

        def attention_rescale(j):
            # partition_broadcast lives on gpsimd: emitted one chunk late so
            # every AllGather trigger is already queued ahead of it
            for u in range(2):
                ycp, rs = resc.pop((j, u))
                bc = att_sb.tile([64, 512], f32, name="bc", tag="bc")
                nc.gpsimd.partition_broadcast(bc, rs)
                nc.vector.tensor_mul(ynT[64 * u:64 * u + 64, j, :], ycp, bc)

        # proj/MLP-era constants: emitted early so their DMAs run during
        # the attention phase
        wp_sb = pR.tile([128, CCH, C], bf16)
        nc.sync.dma_start(out=wp_sb, in_=wp.ap())
        if not trivial_ln2:
            ln2w_bc = pR.tile([128, C], f32)
            nc.sync.dma_start(out=ln2w_bc, in_=_bcast(ln2w.ap()))
            ln2b_bc = pR.tile([128, C], f32)
            nc.sync.dma_start(out=ln2b_bc, in_=_bcast(ln2b.ap()))
        else:
            ln2w_bc = ln2b_bc = None
        if not trivial_b:
            bp_bc = pR.tile([128, C], f32)
            nc.sync.dma_start(out=bp_bc, in_=_bcast(bp.ap()))
        bfc_sb = pR.tile([128, FT], f32)
        nc.sync.dma_start(out=bfc_sb, in_=bf_.ap())
        bm_bc = pR.tile([128, C], f32)
        nc.sync.dma_start(out=bm_bc, in_=_bcast(bm.ap()))
        ident = pR.tile([128, 128], f32)
        nc.sync.dma_start(out=ident, in_=idw.ap())

        chunk(0, True)
        v_in(0)
        attention_main(0)
        chunk(1, False)
        v_in(1)
        attention_rescale(0)
        attention_main(1)
        chunk(2, False)
        v_in(2)
        attention_rescale(1)
        attention_main(2)
        chunk(3, False)
        v_in(3)
        attention_rescale(2)
        attention_main(3)
        attention_rescale(3)
        for j in range(4, PAIRS):
            attention_main(j)
            attention_rescale(j)
        att_ctx.close()
        kvst.close()
        stA.close()

        # ---- attn projection + residual + LN2 + h2^T ----
        pD = st.enter_context(tc.tile_pool(name="pD", bufs=1, side="left"))
        x2 = pD.tile([128, QT, C], f32)
        h2T = pD.tile([128, QT, CCH, 128], bf16)

        with tc.tile_pool(name="ap_ps", bufs=2, space="PSUM") as ap_ps:
            for i in range(QT):
                if not trivial_b:
                    nc.vector.tensor_add(xqs[:, i], xqs[:, i], bp_bc)
                for n in range(C // 512):
                    ps = ap_ps.tile([128, 512], f32, name="ps_a", tag="ps_a")
                    for j in range(PAIRS):
                        nc.tensor.matmul(ps, ynT[:, j, ts(i, 128)],
                                         wp_sb[:, j, ds(512 * n, 512)],
                                         start=(j == 0), stop=(j == PAIRS - 1))
                    nc.vector.tensor_add(x2[:, i, ds(512 * n, 512)], ps,
                                         xqs[:, i, ds(512 * n, 512)])
                h2_t = stream.tile([128, C], bf16, name="h2_t", tag="h_t", bufs=6)
                layer_norm(x2[:, i, :], ln2w_bc, ln2b_bc, h2_t, trivial_ln2)
                nc.sync.dma_start_transpose(h2T[:, i], h2_t[:])
                if i == QT - 1:
                    # prefetch the gelu table while proj/LN2 finish; input is
                    # the LN2 output tile so the scheduler cannot hoist it
                    # before the LN rstds (they share the ACT queue)
                    gd = stat.tile([128, 1], f32, name="gd", bufs=6)
                    nc.scalar.activation(gd, h2_t[:, 0:1], AF.Gelu_apprx_tanh)

        # ---- MLP ----
        # fold the mlp_proj bias into the residual copy while fc runs
        for i in range(QT):
            nc.vector.tensor_add(x2[:, i, :], x2[:, i, :], bm_bc)
        gT = pD.tile([128, FT, TQ], bf16)
        with tc.tile_pool(name="fc_ps", bufs=4, space="PSUM") as fc_ps, \
             tc.tile_pool(name="wf_sb", bufs=6) as wf_pool:
            for t in range(FT):
                wf_t = wf_pool.tile([128, CCH, 128], bf16, name="wf_t", tag="wf_t")
                nc.sync.dma_start(out=wf_t, in_=wf.ap()[t])
                ps = fc_ps.tile([128, 512], f32, name="ps_f", tag="ps_f")
                for c in range(CCH):
                    nc.tensor.matmul(ps, wf_t[:, c, :], h2T[:, 0:QT, c, :],
                                     start=(c == 0), stop=(c == CCH - 1))
                nc.scalar.activation(gT[:, t, :], ps, AF.Gelu_apprx_tanh,
                                     bias=bfc_sb[:, t:t + 1], scale=1.0)

        with tc.tile_pool(name="m_ps", bufs=1, space="PSUM") as m_ps, \
             tc.tile_pool(name="wm_sb", bufs=6) as wm_pool, \
             tc.tile_pool(name="out_sb", bufs=2) as out_pool:
            ps_m = [m_ps.tile([128, 512], f32, name=f"ps_m{k}", tag=f"ps_m{k}")
                    for k in range(8)]
            for i in range(QT):
                nc.tensor.matmul(ps_m[i * 2 + 1], ident,
                                 x2[:, i, ds(512, 512)],
                                 start=True, stop=False)
            for t in range(FT):
                wm_t = wm_pool.tile([128, C], bf16, name="wm_t", tag="wm_t")
                nc.sync.dma_start(out=wm_t, in_=wm.ap()[ts(t, 128), :])
                for i in range(QT):
                    for n in range(C // 512):
                        nc.tensor.matmul(ps_m[i * 2 + n], gT[:, t, ts(i, 128)],
                                         wm_t[:, ds(512 * n, 512)],
                                         start=(t == 0 and n == 0),
                                         stop=(t == FT - 1))
            for i in range(QT):
                out_t = out_pool.tile([128, C], f32, name="out_t", tag="out_t")
                # n==0: residual added here on DVE; n==1: residual was
                # preloaded into PSUM, a scalar-engine copy suffices -- the
                # two run concurrently and each chunk DMAs out immediately
                nc.vector.tensor_add(out_t[:, 0:512], ps_m[i * 2],
                                     x2[:, i, 0:512])
                nc.sync.dma_start(out=out.ap()[ts(i, 128), 0:512],
                                  in_=out_t[:, 0:512])
                nc.scalar.activation(out_t[:, 512:1024], ps_m[i * 2 + 1],
                                     AF.Identity)
                nc.sync.dma_start(out=out.ap()[ts(i, 128), ds(512, 512)],
                                  in_=out_t[:, 512:1024])


def _get_program(trivial_ln1, trivial_ln2, trivial_b):
    key = (trivial_ln1, trivial_ln2, trivial_b)
    if key not in _CACHED:
        _CACHED[key] = _build_program(trivial_ln1, trivial_ln2, trivial_b)
    return _CACHED[key]


def _tile_proj_weight(w):
    # [C, N] f32 -> [128, CCH, N] bf16 with partition = c % 128, chunk = c // 128
    w = np.asarray(w, np.float32).reshape(CCH, 128, -1)
    return np.ascontiguousarray(w.transpose(1, 0, 2).astype(ml_dtypes.bfloat16))


def _prep_in_maps(inputs):
    fl = lambda a: np.ascontiguousarray(np.asarray(a, np.float32))
    x = fl(inputs["x"])
    attn_w = fl(inputs["attn_w"])
    attn_b = fl(inputs["attn_b"])
    wf_full = fl(inputs["fc_w"])  # [C, F]
    # wf tiled: [FT, 128(c), CCH, 128(f')]
    wf_t = wf_full.reshape(CCH, 128, FT, 128).transpose(2, 1, 0, 3)
    wf_t = np.ascontiguousarray(wf_t.astype(ml_dtypes.bfloat16))
    pb = lambda b: np.ascontiguousarray(
        np.asarray(b, np.float32).reshape(-1, 128).T)  # [128, tiles]
    shared = {
        "wq": _tile_proj_weight(attn_w[:, 0:C]),
        "wk": _tile_proj_weight(attn_w[:, C:2 * C]),
        "wv": _tile_proj_weight(attn_w[:, 2 * C:3 * C]),
        "bq": pb(attn_b[0:C]), "bk": pb(attn_b[C:2 * C]),
        "bv": fl(attn_b[2 * C:3 * C]),
        "ln1w": fl(inputs["ln1_w"]), "ln1b": fl(inputs["ln1_b"]),
        "ln2w": fl(inputs["ln2_w"]), "ln2b": fl(inputs["ln2_b"]),
        "wp": _tile_proj_weight(inputs["attn_proj_w"]),
        "bp": fl(inputs["attn_proj_b"]),
        "wf": wf_t, "bf": pb(inputs["fc_b"]),
        "wm": np.ascontiguousarray(fl(inputs["mlp_proj_w"]).astype(ml_dtypes.bfloat16)),
        "bm": fl(inputs["mlp_proj_b"]),
        "idw": np.eye(128, dtype=np.float32),
    }
    in_maps = []
    for core in range(NCORES):
        b, r = core // GROUP, core % GROUP
        in_maps.append({
            "xq": np.ascontiguousarray(x[b, TQ * r:TQ * (r + 1)]),
            **shared,
        })
    return in_maps


def run(inputs, trace=False):
    trivial_ln1 = bool(np.all(np.asarray(inputs["ln1_w"]) == 1.0)
                       and np.all(np.asarray(inputs["ln1_b"]) == 0.0))
    trivial_ln2 = bool(np.all(np.asarray(inputs["ln2_w"]) == 1.0)
                       and np.all(np.asarray(inputs["ln2_b"]) == 0.0))
    trivial_b = bool(np.all(np.asarray(inputs["attn_b"]) == 0.0))
    nc = _get_program(trivial_ln1, trivial_ln2, trivial_b)
    in_maps = _prep_in_maps(inputs)
    res = run_bass_kernel_spmd(nc, in_maps, core_ids=list(range(NCORES)),
                               trace=trace)
    out = np.empty((B, T, C), np.float32)
    for core in range(NCORES):
        b, r = core // GROUP, core % GROUP
        out[b, TQ * r:TQ * (r + 1)] = res.results[core]["out"]
    return out, res


def kernel(**inputs):
    out, _ = run(inputs, trace=False)
    return out


# revision 11
# speedup vs baseline: 1.1158x; 1.0688x over previous
"""Trainium2 Bass kernel for a GPT-2 style transformer block (pre-LN, no mask).

Reference shapes: x [B=2, T=2048, C=1024], H=16 heads, MLP hidden 4C=4096.

Sharding (8 NeuronCores): data-parallel over B (cores 0-3 -> batch 0,
cores 4-7 -> batch 1); within each 4-core group the 2048 rows are split
512 per core. Each core computes LN1 + Q/K/V only for its OWN 512 rows,
then K^T and V are AllGather'd across the 4-core group in fp8e4 (2-pair
chunks, K and V alternating) so attention/proj/MLP stay fully local per
core. A tiny dummy AllGather issued at t=0 absorbs the first-collective
latency (ncfw warmup / core-start skew); its input DMA is issued from
the gpsimd queue so it cannot serialize the sync-engine DMA stream.

Compute layout: activations feeding matmul contractions are kept
feature-major ("transposed", [C, t]) via the DMA xbar transpose; scores
are computed as S^T = K Q^T per head ([tk, tq]) with two heads packed
into the 128-wide contraction via row tiling (the two matmuls run
concurrently in disjoint PE row groups); exp runs on the scalar engine
straight out of PSUM; P @ V uses a [V | ones] stationary operand so the
softmax denominators accumulate in the same PSUM tile as Y^T. The
gathered K/V stay fp8 all the way into the matmuls (stationary fp8 x
moving bf16 is legal on the PE).

LayerNorm rstd is computed as exp(-0.5*ln(var+eps)) so the scalar
engine only ever needs the natural_log_exp table set plus gelu -- two
table loads total, both off the critical path (a dummy gelu during the
proj phase prefetches the gelu set).
"""

import numpy as np
import ml_dtypes

import concourse.bass as bass
import concourse.bacc as bacc
import concourse.tile as tile
from concourse import mybir
from concourse.bass import ts, ds
from concourse.bass_utils import run_bass_kernel_spmd

f32 = mybir.dt.float32
bf16 = mybir.dt.bfloat16
fp8 = mybir.dt.float8e4
AF = mybir.ActivationFunctionType
OP = mybir.AluOpType

B, T, C, H = 2, 2048, 1024, 16
DH = C // H          # 64
F = 4 * C            # 4096
NCORES = 8
GROUP = 4            # cores per batch
TQ = T // GROUP      # 512 own rows per core
NT = T // 128        # 16 key tiles
CCH = C // 128       # 8 contraction chunks over C
PAIRS = H // 2       # 8 head pairs
FT = F // 128        # 32 hidden tiles
QT = TQ // 128       # 4 own-row tiles
VW = DH + 1          # 65: V columns per head incl. ones column
GROUPS = [[0, 1, 2, 3], [4, 5, 6, 7]]

_CACHED = {}


def _bcast(ap, parts=128):
    """DRAM AP for a 1-D tensor broadcast across `parts` partitions."""
    return bass.AP(tensor=ap.tensor, offset=ap.offset, ap=[[0, parts]] + list(ap.ap))


def _build_program(trivial_ln1, trivial_ln2, trivial_b):
    nc = bacc.Bacc("TRN2", target_bir_lowering=False, debug=False,
                   num_devices=NCORES)

    xq = nc.dram_tensor("xq", [TQ, C], f32, kind="ExternalInput")
    # pre-tiled weights: [128 (c within chunk), CCH, out-features]
    wq = nc.dram_tensor("wq", [128, CCH, C], bf16, kind="ExternalInput")
    wk = nc.dram_tensor("wk", [128, CCH, C], bf16, kind="ExternalInput")
    wv = nc.dram_tensor("wv", [128, CCH, C], bf16, kind="ExternalInput")
    bqv = nc.dram_tensor("bq", [128, PAIRS], f32, kind="ExternalInput")
    bkv = nc.dram_tensor("bk", [128, PAIRS], f32, kind="ExternalInput")
    bvv = nc.dram_tensor("bv", [C], f32, kind="ExternalInput")
    ln1w = nc.dram_tensor("ln1w", [C], f32, kind="ExternalInput")
    ln1b = nc.dram_tensor("ln1b", [C], f32, kind="ExternalInput")
    ln2w = nc.dram_tensor("ln2w", [C], f32, kind="ExternalInput")
    ln2b = nc.dram_tensor("ln2b", [C], f32, kind="ExternalInput")
    wp = nc.dram_tensor("wp", [128, CCH, C], bf16, kind="ExternalInput")
    bp = nc.dram_tensor("bp", [C], f32, kind="ExternalInput")
    # wf pre-tiled per f'-tile: [FT, 128 (c), CCH, 128 (f')]
    wf = nc.dram_tensor("wf", [FT, 128, CCH, 128], bf16, kind="ExternalInput")
    bf_ = nc.dram_tensor("bf", [128, FT], f32, kind="ExternalInput")
    wm = nc.dram_tensor("wm", [F, C], bf16, kind="ExternalInput")
    bm = nc.dram_tensor("bm", [C], f32, kind="ExternalInput")
    idw = nc.dram_tensor("idw", [128, 128], f32, kind="ExternalInput")
    out = nc.dram_tensor("out", [TQ, C], f32, kind="ExternalOutput")

    # collective scratch (Internal DRAM); chunk a covers head pairs 2a,2a+1
    ko = [nc.dram_tensor(f"ko{a}", [256, TQ], fp8, kind="Internal")
          for a in range(4)]
    kg = [nc.dram_tensor(f"kg{a}", [GROUP, 256, TQ], fp8, kind="Internal")
          for a in range(4)]
    vo = [nc.dram_tensor(f"vo{a}", [TQ, 4 * VW], fp8, kind="Internal")
          for a in range(4)]
    vg = [nc.dram_tensor(f"vg{a}", [GROUP, TQ, 4 * VW], fp8, kind="Internal")
          for a in range(4)]
    din = nc.dram_tensor("din", [64], bf16, kind="Internal")
    dout = nc.dram_tensor("dout", [GROUP * 64], bf16, kind="Internal")

    with tile.TileContext(nc) as tc:
        _emit(nc, tc, trivial_ln1, trivial_ln2, trivial_b,
              xq, wq, wk, wv, bqv, bkv, bvv, ln1w, ln1b, ln2w, ln2b,
              wp, bp, wf, bf_, wm, bm, idw, out,
              ko, kg, vo, vg, din, dout)
    nc.compile()
    return nc


def _emit(nc, tc, trivial_ln1, trivial_ln2, trivial_b,
          xq, wq, wk, wv, bqv, bkv, bvv, ln1w, ln1b, ln2w, ln2b,
          wp, bp, wf, bf_, wm, bm, idw, out,
          ko, kg, vo, vg, din, dout):
    from contextlib import ExitStack

    with ExitStack() as st:
        persist = st.enter_context(tc.tile_pool(name="persist", bufs=1))
        stat = st.enter_context(tc.tile_pool(name="stat", bufs=4))
        stream = st.enter_context(tc.tile_pool(name="stream", bufs=4))

        eps_t = persist.tile([128, 1], f32)
        nc.vector.memset(eps_t, 1e-5)

        # ---------------- pools ----------------
        stA = st.enter_context(ExitStack())
        pA = stA.enter_context(tc.tile_pool(name="pA", bufs=1, side="left"))
        pR = st.enter_context(tc.tile_pool(name="pR", bufs=1, side="right"))

        # persistent activations
        hT = pA.tile([128, QT, CCH, 128], bf16)
        xqs = pR.tile([128, QT, C], f32)          # own x rows (LN1 + residual)
        qT = pR.tile([128, PAIRS, TQ], bf16)
        v_sb = pR.tile([128, NT, H * VW], fp8)    # [tok, tile, 16*(DH+1)]
        ynT = pR.tile([128, PAIRS, TQ], bf16)

        # x rows first in the DMA queues (everything hangs off LN1)
        for i in range(QT):
            nc.sync.dma_start(out=xqs[:, i], in_=xq.ap()[ts(i, 128), :])
        wk_sb = pA.tile([128, CCH, C], bf16)
        nc.sync.dma_start(out=wk_sb, in_=wk.ap())
        wq_sb = pA.tile([128, CCH, C], bf16)
        nc.sync.dma_start(out=wq_sb, in_=wq.ap())
        wv_sb = pA.tile([128, CCH, C], bf16)
        nc.sync.dma_start(out=wv_sb, in_=wv.ap())
        if not trivial_b:
            bq_sb = pA.tile([128, PAIRS], f32)
            nc.sync.dma_start(out=bq_sb, in_=bqv.ap())
            bk_sb = pA.tile([128, PAIRS], f32)
            nc.sync.dma_start(out=bk_sb, in_=bkv.ap())
            bv_bc = pA.tile([128, C], f32)
            nc.sync.dma_start(out=bv_bc, in_=_bcast(bvv.ap()))
        if not trivial_ln1:
            ln1w_bc = pA.tile([128, C], f32)
            nc.sync.dma_start(out=ln1w_bc, in_=_bcast(ln1w.ap()))
            ln1b_bc = pA.tile([128, C], f32)
            nc.sync.dma_start(out=ln1b_bc, in_=_bcast(ln1b.ap()))
        else:
            ln1w_bc = ln1b_bc = None

        def layer_norm(x_t, w_bc, b_bc, out_ap, trivial):
            """x_t [128, C] f32 -> out_ap [128, C] bf16 (normalized + affine).

            rstd = exp(-0.5 * ln(var + eps)) keeps ACT on the ln/exp table."""
            stats = stat.tile([128, 2, nc.vector.BN_STATS_DIM], f32, name="stats", bufs=6)
            nc.vector.bn_stats(out=stats[:, 0, :], in_=x_t[:, 0:512])
            nc.vector.bn_stats(out=stats[:, 1, :], in_=x_t[:, 512:1024])
            mv = stat.tile([128, nc.vector.BN_AGGR_DIM], f32, name="mv", bufs=6)
            nc.vector.bn_aggr(out=mv, in_=stats)
            rstd = stat.tile([128, 1], f32, name="rstd", bufs=6)
            nc.scalar.activation(rstd, mv[:, 1:2], AF.Sqrt, bias=eps_t)
            nc.vector.reciprocal(rstd, rstd)
            if trivial:
                nc.vector.tensor_scalar(out=out_ap, in0=x_t, scalar1=mv[:, 0:1],
                                        scalar2=rstd, op0=OP.subtract, op1=OP.mult)
            else:
                t1 = stat.tile([128, C], f32, name="t1", tag="ln_t1")
                nc.vector.tensor_scalar(out=t1, in0=x_t, scalar1=mv[:, 0:1],
                                        scalar2=rstd, op0=OP.subtract, op1=OP.mult)
                nc.vector.tensor_mul(t1, t1, w_bc)
                nc.vector.tensor_add(out_ap, t1, b_bc)

        # ---- LN1 over own 4 tiles ----
        for i in range(QT):
            h_t = stream.tile([128, C], bf16, name="h_t", tag="h_t", bufs=6)
            layer_norm(xqs[:, i], ln1w_bc, ln1b_bc, h_t, trivial_ln1)
            nc.sync.dma_start_transpose(hT[:, i], h_t[:])

        # ---- own K/Q/V per 2-pair chunk, AllGathers fired ASAP ----
        kvst = st.enter_context(ExitStack())
        kv_ps = kvst.enter_context(tc.tile_pool(name="kv_ps", bufs=2, space="PSUM"))
        vos_pool = kvst.enter_context(tc.tile_pool(name="vos", bufs=2))

        def chunk(a, on_act):
            # K for pairs 2a, 2a+1  -> AG;  Q same pairs;  V heads 4a..4a+4 -> AG
            for e in range(2):
                j = 2 * a + e
                ps = kv_ps.tile([128, TQ], f32, name="ps_k", tag="ps_kv")
                for c in range(CCH):
                    nc.tensor.matmul(ps, wk_sb[:, c, ts(j, 128)],
                                     hT[:, 0:QT, c, :],
                                     start=(c == 0), stop=(c == CCH - 1))
                kt_t = stream.tile([128, TQ], fp8, name="kt_t", tag="kt_t", bufs=4)
                if trivial_b:
                    if on_act:
                        nc.scalar.activation(kt_t, ps, AF.Identity)
                    else:
                        nc.vector.tensor_copy(kt_t, ps)
                else:
                    nc.vector.tensor_scalar(out=kt_t, in0=ps,
                                            scalar1=bk_sb[:, j:j + 1],
                                            scalar2=None, op0=OP.add)
                nc.sync.dma_start(out=ko[a].ap()[ts(e, 128), :], in_=kt_t)
            nc.gpsimd.collective_compute(
                "AllGather", OP.bypass, replica_groups=GROUPS,
                ins=[ko[a].ap().opt()], outs=[kg[a].ap().opt()])
            for e in range(2):
                j = 2 * a + e
                ps = kv_ps.tile([128, TQ], f32, name="ps_q", tag="ps_kv")
                for c in range(CCH):
                    nc.tensor.matmul(ps, wq_sb[:, c, ts(j, 128)],
                                     hT[:, 0:QT, c, :],
                                     start=(c == 0), stop=(c == CCH - 1))
                if trivial_b:
                    if on_act:
                        nc.scalar.activation(qT[:, j], ps, AF.Identity)
                    else:
                        nc.vector.tensor_copy(qT[:, j], ps)
                else:
                    nc.vector.tensor_scalar(out=qT[:, j], in0=ps,
                                            scalar1=bq_sb[:, j:j + 1],
                                            scalar2=None, op0=OP.add)
            vos = vos_pool.tile([128, QT, 4 * VW], fp8, name="vos", tag="vos")
            vosv = vos.rearrange("p q (h x) -> p q h x", x=VW)
            nc.vector.memset(vosv[:, :, :, DH:DH + 1], 1.0)
            for i in range(QT):
                ps = kv_ps.tile([128, TQ], f32, name="ps_v", tag="ps_kv")
                for c in range(CCH):
                    nc.tensor.matmul(ps[:, 0:256], hT[:, i, c, :],
                                     wv_sb[:, c, ds(256 * a, 256)],
                                     start=(c == 0), stop=(c == CCH - 1))
                dst = vosv[:, i, :, 0:DH]
                psv = ps[:, 0:256].rearrange("p (h x) -> p h x", x=DH)
                if trivial_b:
                    if on_act:
                        nc.scalar.activation(dst, psv, AF.Identity)
                    else:
                        nc.vector.tensor_copy(dst, psv)
                else:
                    bvw = bv_bc[:, ds(256 * a, 256)].rearrange(
                        "p (h x) -> p h x", x=DH)
                    nc.vector.tensor_add(dst, psv, bvw)
            nc.sync.dma_start(
                out=vo[a].ap().rearrange("(q p) f -> p q f", p=128), in_=vos)
            nc.gpsimd.collective_compute(
                "AllGather", OP.bypass, replica_groups=GROUPS,
                ins=[vo[a].ap().opt()], outs=[vg[a].ap().opt()])

        def v_in(a):
            # gathered V chunk -> resident v_sb, per key tile
            for t in range(NT):
                r, w = t // 4, t % 4
                nc.sync.dma_start(
                    out=v_sb[:, t, ds(4 * VW * a, 4 * VW)],
                    in_=vg[a].ap()[r, ts(w, 128), :])

        # ---- attention (kT/V from the AllGathers, fp8 stationaries) ----
        scale = 1.0 / float(np.sqrt(DH))
        att_ctx = st.enter_context(ExitStack())
        s_ps = att_ctx.enter_context(tc.tile_pool(name="s_ps", bufs=2, space="PSUM"))
        y_ps = att_ctx.enter_context(tc.tile_pool(name="y_ps", bufs=1, space="PSUM"))
        kq_sb = att_ctx.enter_context(tc.tile_pool(name="kq_sb", bufs=2))
        att_sb = att_ctx.enter_context(tc.tile_pool(name="att_sb", bufs=3))
        resc = {}

        def attention_main(j):
            a, e = j // 2, j % 2
            kT_j = kq_sb.tile([128, T], fp8, name="kT_j", tag="kT_j")
            for r in range(GROUP):
                nc.sync.dma_start(out=kT_j[:, ds(TQ * r, TQ)],
                                  in_=kg[a].ap()[r, ts(e, 128), :])
            ps_y1 = y_ps.tile([VW, 512], f32, name="ps_y1", tag="ps_y1")
            ps_y2 = y_ps.tile([VW, 512], f32, name="ps_y2", tag="ps_y2")
            for cidx in range(NT):
                ps_s = s_ps.tile([128, 1024], f32, name="ps_s", tag="ps_s")
                nc.tensor.matmul(ps_s[:, 0:512],
                                 kT_j[0:64, ts(cidx, 128)],
                                 qT[0:64, j, :], start=True, stop=True)
                nc.tensor.matmul(ps_s[:, 512:1024],
                                 kT_j[64:128, ts(cidx, 128)],
                                 qT[64:128, j, :], start=True, stop=True,
                                 tile_position=(64, 0))
                pT = att_sb.tile([128, 1024], bf16, name="pT", tag="pT")
                nc.scalar.activation(pT, ps_s, AF.Exp, scale=scale)
                nc.tensor.matmul(ps_y1,
                                 v_sb[:, cidx, ds(VW * 2 * j, VW)],
                                 pT[:, 0:512],
                                 start=(cidx == 0), stop=(cidx == NT - 1))
                nc.tensor.matmul(ps_y2,
                                 v_sb[:, cidx, ds(VW * (2 * j + 1), VW)],
                                 pT[:, 512:1024],
                                 start=(cidx == 0), stop=(cidx == NT - 1))
            # copy Y and the sums row out of PSUM right away so the
            # accumulator banks free up for the next pair; the sums
            # staging copy also moves them to SBUF partition 0
            # (custom-DVE ops mis-read PSUM at a partition offset)
            for u, ps_y in ((0, ps_y1), (1, ps_y2)):
                ycp = att_sb.tile([64, 512], f32, name="ycp", tag=f"ycp{u}", bufs=2)
                nc.vector.tensor_copy(ycp, ps_y[0:DH, :])
                rs0 = att_sb.tile([1, 512], f32, name="rs0", tag=f"rs0{u}", bufs=2)
                nc.vector.tensor_copy(rs0, ps_y[DH:DH + 1, :])
                rs = att_sb.tile([1, 512], f32, name="rs", tag=f"rs{u}", bufs=2)
                nc.vector.reciprocal_approx_fast(rs, rs0)
                resc[(j, u)] = (ycp, rs)

        def attention_rescale(j):
            # partition_broadcast lives on gpsimd: emitted one chunk late so
            # every AllGather trigger is already queued ahead of it
            for u in range(2):
                ycp, rs = resc.pop((j, u))
                bc = att_sb.tile([64, 512], f32, name="bc", tag="bc")
                nc.gpsimd.partition_broadcast(bc, rs)
                nc.vector.tensor_mul(ynT[64 * u:64 * u + 64, j, :], ycp, bc)

        # proj/MLP-era constants: emitted early so their DMAs run during
        # the attention phase
        wp_sb = pR.tile([128, CCH, C], bf16)
        nc.sync.dma_start(out=wp_sb, in_=wp.ap())
        if not trivial_ln2:
            ln2w_bc = pR.tile([128, C], f32)
            nc.sync.dma_start(out=ln2w_bc, in_=_bcast(ln2w.ap()))
            ln2b_bc = pR.tile([128, C], f32)
            nc.sync.dma_start(out=ln2b_bc, in_=_bcast(ln2b.ap()))
        else:
            ln2w_bc = ln2b_bc = None
        if not trivial_b:
            bp_bc = pR.tile([128, C], f32)
            nc.sync.dma_start(out=bp_bc, in_=_bcast(bp.ap()))
        bfc_sb = pR.tile([128, FT], f32)
        nc.sync.dma_start(out=bfc_sb, in_=bf_.ap())
        bm_bc = pR.tile([128, C], f32)
        nc.sync.dma_start(out=bm_bc, in_=_bcast(bm.ap()))
        ident = pR.tile([128, 128], f32)
        nc.sync.dma_start(out=ident, in_=idw.ap())

        # dummy warm-up collective: no inputs, result unused. Emitted after
        # the LN1/weight/transpose DMAs (so their semaphores never recycle
        # through its completion sem) but before the first real AllGather,
        # absorbing the ~40us ncfw first-collective warmup / core skew.
        nc.gpsimd.collective_compute(
            "AllGather", OP.bypass, replica_groups=GROUPS,
            ins=[din.ap().opt()], outs=[dout.ap().opt()])

        chunk(0, True)
        v_in(0)
        attention_main(0)
        chunk(1, False)
        v_in(1)
        attention_rescale(0)
        attention_main(1)
        chunk(2, False)
        v_in(2)
        attention_rescale(1)
        attention_main(2)
        chunk(3, False)
        v_in(3)
        attention_rescale(2)
        attention_main(3)
        attention_rescale(3)
        for j in range(4, PAIRS):
            attention_main(j)
            attention_rescale(j)
        att_ctx.close()
        kvst.close()
        stA.close()

        # ---- attn projection + residual + LN2 + h2^T ----
        pD = st.enter_context(tc.tile_pool(name="pD", bufs=1, side="left"))
        x2 = pD.tile([128, QT, C], f32)
        h2T = pD.tile([128, QT, CCH, 128], bf16)

        with tc.tile_pool(name="ap_ps", bufs=2, space="PSUM") as ap_ps:
            for i in range(QT):
                if not trivial_b:
                    nc.vector.tensor_add(xqs[:, i], xqs[:, i], bp_bc)
                for n in range(C // 512):
                    ps = ap_ps.tile([128, 512], f32, name="ps_a", tag="ps_a")
                    for j in range(PAIRS):
                        nc.tensor.matmul(ps, ynT[:, j, ts(i, 128)],
                                         wp_sb[:, j, ds(512 * n, 512)],
                                         start=(j == 0), stop=(j == PAIRS - 1))
                    nc.vector.tensor_add(x2[:, i, ds(512 * n, 512)], ps,
                                         xqs[:, i, ds(512 * n, 512)])
                h2_t = stream.tile([128, C], bf16, name="h2_t", tag="h_t", bufs=6)
                layer_norm(x2[:, i, :], ln2w_bc, ln2b_bc, h2_t, trivial_ln2)
                nc.sync.dma_start_transpose(h2T[:, i], h2_t[:])
                if i == QT - 1:
                    # prefetch the gelu table while proj/LN2 finish; input is
                    # the LN2 output tile so the scheduler cannot hoist it
                    # before the LN rstds (they share the ACT queue)
                    gd = stat.tile([128, 1], f32, name="gd", bufs=6)
                    nc.scalar.activation(gd, h2_t[:, 0:1], AF.Gelu_apprx_tanh)

        # ---- MLP ----
        # fold the mlp_proj bias into the residual copy while fc runs
        for i in range(QT):
            nc.vector.tensor_add(x2[:, i, :], x2[:, i, :], bm_bc)
        gT = pD.tile([128, FT, TQ], bf16)
        with tc.tile_pool(name="fc_ps", bufs=4, space="PSUM") as fc_ps, \
             tc.tile_pool(name="wf_sb", bufs=6) as wf_pool:
            wf_tiles = {}
            for t in range(6):
                wf_tiles[t] = wf_pool.tile([128, CCH, 128], bf16,
                                           name="wf_t", tag="wf_t")
                nc.sync.dma_start(out=wf_tiles[t], in_=wf.ap()[t])
            for t in range(FT):
                if t in wf_tiles:
                    wf_t = wf_tiles.pop(t)
                else:
                    wf_t = wf_pool.tile([128, CCH, 128], bf16, name="wf_t",
                                        tag="wf_t")
                    nc.sync.dma_start(out=wf_t, in_=wf.ap()[t])
                ps = fc_ps.tile([128, 512], f32, name="ps_f", tag="ps_f")
                for c in range(CCH):
                    nc.tensor.matmul(ps, wf_t[:, c, :], h2T[:, 0:QT, c, :],
                                     start=(c == 0), stop=(c == CCH - 1))
                nc.scalar.activation(gT[:, t, :], ps, AF.Gelu_apprx_tanh,
                                     bias=bfc_sb[:, t:t + 1], scale=1.0)

        with tc.tile_pool(name="m_ps", bufs=1, space="PSUM") as m_ps, \
             tc.tile_pool(name="wm_sb", bufs=6) as wm_pool, \
             tc.tile_pool(name="out_sb", bufs=2) as out_pool:
            ps_m = [m_ps.tile([128, 512], f32, name=f"ps_m{k}", tag=f"ps_m{k}")
                    for k in range(8)]
            for i in range(QT):
                nc.tensor.matmul(ps_m[i * 2 + 1], ident,
                                 x2[:, i, ds(512, 512)],
                                 start=True, stop=False)
            for t in range(FT):
                wm_t = wm_pool.tile([128, C], bf16, name="wm_t", tag="wm_t")
                nc.sync.dma_start(out=wm_t, in_=wm.ap()[ts(t, 128), :])
                for i in range(QT):
                    for n in range(C // 512):
                        nc.tensor.matmul(ps_m[i * 2 + n], gT[:, t, ts(i, 128)],
                                         wm_t[:, ds(512 * n, 512)],
                                         start=(t == 0 and n == 0),
                                         stop=(t == FT - 1))
            for i in range(QT):
                out_t = out_pool.tile([128, C], f32, name="out_t", tag="out_t")
                # n==0: residual added here on DVE; n==1: residual was
                # preloaded into PSUM, a scalar-engine copy suffices -- the
                # two run concurrently and each chunk DMAs out immediately
                nc.vector.tensor_add(out_t[:, 0:512], ps_m[i * 2],
                                     x2[:, i, 0:512])
                nc.sync.dma_start(out=out.ap()[ts(i, 128), 0:512],
                                  in_=out_t[:, 0:512])
                nc.scalar.activation(out_t[:, 512:1024], ps_m[i * 2 + 1],
                                     AF.Identity)
                nc.sync.dma_start(out=out.ap()[ts(i, 128), ds(512, 512)],
                                  in_=out_t[:, 512:1024])


def _get_program(trivial_ln1, trivial_ln2, trivial_b):
    key = (trivial_ln1, trivial_ln2, trivial_b)
    if key not in _CACHED:
        _CACHED[key] = _build_program(trivial_ln1, trivial_ln2, trivial_b)
    return _CACHED[key]


def _tile_proj_weight(w):
    # [C, N] f32 -> [128, CCH, N] bf16 with partition = c % 128, chunk = c // 128
    w = np.asarray(w, np.float32).reshape(CCH, 128, -1)
    return np.ascontiguousarray(w.transpose(1, 0, 2).astype(ml_dtypes.bfloat16))


def _prep_in_maps(inputs):
    fl = lambda a: np.ascontiguousarray(np.asarray(a, np.float32))
    x = fl(inputs["x"])
    attn_w = fl(inputs["attn_w"])
    attn_b = fl(inputs["attn_b"])
    wf_full = fl(inputs["fc_w"])  # [C, F]
    # wf tiled: [FT, 128(c), CCH, 128(f')]
    wf_t = wf_full.reshape(CCH, 128, FT, 128).transpose(2, 1, 0, 3)
    wf_t = np.ascontiguousarray(wf_t.astype(ml_dtypes.bfloat16))
    pb = lambda b: np.ascontiguousarray(
        np.asarray(b, np.float32).reshape(-1, 128).T)  # [128, tiles]
    shared = {
        "wq": _tile_proj_weight(attn_w[:, 0:C]),
        "wk": _tile_proj_weight(attn_w[:, C:2 * C]),
        "wv": _tile_proj_weight(attn_w[:, 2 * C:3 * C]),
        "bq": pb(attn_b[0:C]), "bk": pb(attn_b[C:2 * C]),
        "bv": fl(attn_b[2 * C:3 * C]),
        "ln1w": fl(inputs["ln1_w"]), "ln1b": fl(inputs["ln1_b"]),
        "ln2w": fl(inputs["ln2_w"]), "ln2b": fl(inputs["ln2_b"]),
        "wp": _tile_proj_weight(inputs["attn_proj_w"]),
        "bp": fl(inputs["attn_proj_b"]),
        "wf": wf_t, "bf": pb(inputs["fc_b"]),
        "wm": np.ascontiguousarray(fl(inputs["mlp_proj_w"]).astype(ml_dtypes.bfloat16)),
        "bm": fl(inputs["mlp_proj_b"]),
        "idw": np.eye(128, dtype=np.float32),
    }
    in_maps = []
    for core in range(NCORES):
        b, r = core // GROUP, core % GROUP
        in_maps.append({
            "xq": np.ascontiguousarray(x[b, TQ * r:TQ * (r + 1)]),
            **shared,
        })
    return in_maps


def run(inputs, trace=False):
    trivial_ln1 = bool(np.all(np.asarray(inputs["ln1_w"]) == 1.0)
                       and np.all(np.asarray(inputs["ln1_b"]) == 0.0))
    trivial_ln2 = bool(np.all(np.asarray(inputs["ln2_w"]) == 1.0)
                       and np.all(np.asarray(inputs["ln2_b"]) == 0.0))
    trivial_b = bool(np.all(np.asarray(inputs["attn_b"]) == 0.0))
    nc = _get_program(trivial_ln1, trivial_ln2, trivial_b)
    in_maps = _prep_in_maps(inputs)
    res = run_bass_kernel_spmd(nc, in_maps, core_ids=list(range(NCORES)),
                               trace=trace)
    out = np.empty((B, T, C), np.float32)
    for core in range(NCORES):
        b, r = core // GROUP, core % GROUP
        out[b, TQ * r:TQ * (r + 1)] = res.results[core]["out"]
    return out, res


def kernel(**inputs):
    out, _ = run(inputs, trace=False)
    return out


# revision 12
# speedup vs baseline: 1.1486x; 1.0294x over previous
"""Trainium2 Bass kernel for a GPT-2 style transformer block (pre-LN, no mask).

Reference shapes: x [B=2, T=2048, C=1024], H=16 heads, MLP hidden 4C=4096.

Sharding (8 NeuronCores): data-parallel over B (cores 0-3 -> batch 0,
cores 4-7 -> batch 1); within each 4-core group the 2048 rows are split
512 per core. Each core computes LN1 + Q/K/V only for its OWN 512 rows,
then K^T and V are AllGather'd across the 4-core group in fp8e4 (2-pair
chunks, K and V alternating) so attention/proj/MLP stay fully local per
core. The first AllGather is emitted after all early DMA/transpose
traffic so the scheduler's semaphore recycling never couples the local
LN1/QKV pipeline to collective completion: each core's own work runs
during the first-collective warmup / core-start skew window.

Compute layout: activations feeding matmul contractions are kept
feature-major ("transposed", [C, t]) via the DMA xbar transpose; scores
are computed as S^T = K Q^T per head ([tk, tq]) with two heads packed
into the 128-wide contraction via row tiling (the two matmuls run
concurrently in disjoint PE row groups); exp runs on the scalar engine
straight out of PSUM; P @ V uses a [V | ones] stationary operand so the
softmax denominators accumulate in the same PSUM tile as Y^T. The
gathered K/V stay fp8 all the way into the matmuls (stationary fp8 x
moving bf16 is legal on the PE).

A dependency-pinned dummy gelu during the proj phase prefetches the
gelu table set so the MLP's first real gelu doesn't stall on the table
load.
"""

import numpy as np
import ml_dtypes

import concourse.bass as bass
import concourse.bacc as bacc
import concourse.tile as tile
from concourse import mybir
from concourse.bass import ts, ds
from concourse.bass_utils import run_bass_kernel_spmd

f32 = mybir.dt.float32
bf16 = mybir.dt.bfloat16
fp8 = mybir.dt.float8e4
AF = mybir.ActivationFunctionType
OP = mybir.AluOpType

B, T, C, H = 2, 2048, 1024, 16
DH = C // H          # 64
F = 4 * C            # 4096
NCORES = 8
GROUP = 4            # cores per batch
TQ = T // GROUP      # 512 own rows per core
NT = T // 128        # 16 key tiles
CCH = C // 128       # 8 contraction chunks over C
PAIRS = H // 2       # 8 head pairs
FT = F // 128        # 32 hidden tiles
QT = TQ // 128       # 4 own-row tiles
VW = DH + 1          # 65: V columns per head incl. ones column
GROUPS = [[0, 1, 2, 3], [4, 5, 6, 7]]

_CACHED = {}


def _bcast(ap, parts=128):
    """DRAM AP for a 1-D tensor broadcast across `parts` partitions."""
    return bass.AP(tensor=ap.tensor, offset=ap.offset, ap=[[0, parts]] + list(ap.ap))


def _build_program(trivial_ln1, trivial_ln2, trivial_b):
    nc = bacc.Bacc("TRN2", target_bir_lowering=False, debug=False,
                   num_devices=NCORES)

    xq = nc.dram_tensor("xq", [TQ, C], f32, kind="ExternalInput")
    # pre-tiled weights: [128 (c within chunk), CCH, out-features]
    wq = nc.dram_tensor("wq", [128, CCH, C], bf16, kind="ExternalInput")
    wk = nc.dram_tensor("wk", [128, CCH, C], bf16, kind="ExternalInput")
    wv = nc.dram_tensor("wv", [128, CCH, C], bf16, kind="ExternalInput")
    bqv = nc.dram_tensor("bq", [128, PAIRS], f32, kind="ExternalInput")
    bkv = nc.dram_tensor("bk", [128, PAIRS], f32, kind="ExternalInput")
    bvv = nc.dram_tensor("bv", [C], f32, kind="ExternalInput")
    ln1w = nc.dram_tensor("ln1w", [C], f32, kind="ExternalInput")
    ln1b = nc.dram_tensor("ln1b", [C], f32, kind="ExternalInput")
    ln2w = nc.dram_tensor("ln2w", [C], f32, kind="ExternalInput")
    ln2b = nc.dram_tensor("ln2b", [C], f32, kind="ExternalInput")
    wp = nc.dram_tensor("wp", [128, CCH, C], bf16, kind="ExternalInput")
    bp = nc.dram_tensor("bp", [C], f32, kind="ExternalInput")
    # wf pre-tiled per f'-tile: [FT, 128 (c), CCH, 128 (f')]
    wf = nc.dram_tensor("wf", [FT, 128, CCH, 128], bf16, kind="ExternalInput")
    bf_ = nc.dram_tensor("bf", [128, FT], f32, kind="ExternalInput")
    wm = nc.dram_tensor("wm", [F, C], bf16, kind="ExternalInput")
    bm = nc.dram_tensor("bm", [C], f32, kind="ExternalInput")
    idw = nc.dram_tensor("idw", [128, 128], f32, kind="ExternalInput")
    out = nc.dram_tensor("out", [TQ, C], f32, kind="ExternalOutput")

    # collective scratch (Internal DRAM); chunk a covers head pairs 2a,2a+1
    ko = [nc.dram_tensor(f"ko{a}", [256, TQ], fp8, kind="Internal")
          for a in range(4)]
    kg = [nc.dram_tensor(f"kg{a}", [GROUP, 256, TQ], fp8, kind="Internal")
          for a in range(4)]
    vo = [nc.dram_tensor(f"vo{a}", [TQ, 4 * VW], fp8, kind="Internal")
          for a in range(4)]
    vg = [nc.dram_tensor(f"vg{a}", [GROUP, TQ, 4 * VW], fp8, kind="Internal")
          for a in range(4)]

    with tile.TileContext(nc) as tc:
        _emit(nc, tc, trivial_ln1, trivial_ln2, trivial_b,
              xq, wq, wk, wv, bqv, bkv, bvv, ln1w, ln1b, ln2w, ln2b,
              wp, bp, wf, bf_, wm, bm, idw, out,
              ko, kg, vo, vg)
    nc.compile()
    return nc


def _emit(nc, tc, trivial_ln1, trivial_ln2, trivial_b,
          xq, wq, wk, wv, bqv, bkv, bvv, ln1w, ln1b, ln2w, ln2b,
          wp, bp, wf, bf_, wm, bm, idw, out,
          ko, kg, vo, vg):
    from contextlib import ExitStack

    with ExitStack() as st:
        persist = st.enter_context(tc.tile_pool(name="persist", bufs=1))
        stat = st.enter_context(tc.tile_pool(name="stat", bufs=4))
        stream = st.enter_context(tc.tile_pool(name="stream", bufs=4))

        eps_t = persist.tile([128, 1], f32)
        nc.vector.memset(eps_t, 1e-5)

        # ---------------- pools ----------------
        stA = st.enter_context(ExitStack())
        pA = stA.enter_context(tc.tile_pool(name="pA", bufs=1, side="left"))
        pR = st.enter_context(tc.tile_pool(name="pR", bufs=1, side="right"))

        # persistent activations
        hT = pA.tile([128, QT, CCH, 128], bf16)
        xqs = pR.tile([128, QT, C], f32)          # own x rows (LN1 + residual)
        qT = pR.tile([128, PAIRS, TQ], bf16)
        v_sb = pR.tile([128, NT, H * VW], fp8)    # [tok, tile, 16*(DH+1)]
        ynT = pR.tile([128, PAIRS, TQ], bf16)

        # x rows first in the DMA queues (everything hangs off LN1)
        for i in range(QT):
            nc.sync.dma_start(out=xqs[:, i], in_=xq.ap()[ts(i, 128), :])
        wk_sb = pA.tile([128, CCH, C], bf16)
        nc.sync.dma_start(out=wk_sb, in_=wk.ap())
        wq_sb = pA.tile([128, CCH, C], bf16)
        nc.sync.dma_start(out=wq_sb, in_=wq.ap())
        wv_sb = pA.tile([128, CCH, C], bf16)
        nc.sync.dma_start(out=wv_sb, in_=wv.ap())
        if not trivial_b:
            bq_sb = pA.tile([128, PAIRS], f32)
            nc.sync.dma_start(out=bq_sb, in_=bqv.ap())
            bk_sb = pA.tile([128, PAIRS], f32)
            nc.sync.dma_start(out=bk_sb, in_=bkv.ap())
            bv_bc = pA.tile([128, C], f32)
            nc.sync.dma_start(out=bv_bc, in_=_bcast(bvv.ap()))
        if not trivial_ln1:
            ln1w_bc = pA.tile([128, C], f32)
            nc.sync.dma_start(out=ln1w_bc, in_=_bcast(ln1w.ap()))
            ln1b_bc = pA.tile([128, C], f32)
            nc.sync.dma_start(out=ln1b_bc, in_=_bcast(ln1b.ap()))
        else:
            ln1w_bc = ln1b_bc = None

        def layer_norm(x_t, w_bc, b_bc, out_ap, trivial):
            """x_t [128, C] f32 -> out_ap [128, C] bf16 (normalized + affine).

            rstd = exp(-0.5 * ln(var + eps)) keeps ACT on the ln/exp table."""
            stats = stat.tile([128, 2, nc.vector.BN_STATS_DIM], f32, name="stats", bufs=6)
            nc.vector.bn_stats(out=stats[:, 0, :], in_=x_t[:, 0:512])
            nc.vector.bn_stats(out=stats[:, 1, :], in_=x_t[:, 512:1024])
            mv = stat.tile([128, nc.vector.BN_AGGR_DIM], f32, name="mv", bufs=6)
            nc.vector.bn_aggr(out=mv, in_=stats)
            rstd = stat.tile([128, 1], f32, name="rstd", bufs=6)
            nc.scalar.activation(rstd, mv[:, 1:2], AF.Sqrt, bias=eps_t)
            nc.vector.reciprocal(rstd, rstd)
            if trivial:
                nc.vector.tensor_scalar(out=out_ap, in0=x_t, scalar1=mv[:, 0:1],
                                        scalar2=rstd, op0=OP.subtract, op1=OP.mult)
            else:
                t1 = stat.tile([128, C], f32, name="t1", tag="ln_t1")
                nc.vector.tensor_scalar(out=t1, in0=x_t, scalar1=mv[:, 0:1],
                                        scalar2=rstd, op0=OP.subtract, op1=OP.mult)
                nc.vector.tensor_mul(t1, t1, w_bc)
                nc.vector.tensor_add(out_ap, t1, b_bc)

        # ---- LN1 over own 4 tiles ----
        for i in range(QT):
            h_t = stream.tile([128, C], bf16, name="h_t", tag="h_t", bufs=6)
            layer_norm(xqs[:, i], ln1w_bc, ln1b_bc, h_t, trivial_ln1)
            nc.sync.dma_start_transpose(hT[:, i], h_t[:])

        # ---- own K/Q/V per 2-pair chunk, AllGathers fired ASAP ----
        kvst = st.enter_context(ExitStack())
        kv_ps = kvst.enter_context(tc.tile_pool(name="kv_ps", bufs=2, space="PSUM"))
        vos_pool = kvst.enter_context(tc.tile_pool(name="vos", bufs=2))

        def chunk(a, on_act):
            # K for pairs 2a, 2a+1  -> AG;  Q same pairs;  V heads 4a..4a+4 -> AG
            for e in range(2):
                j = 2 * a + e
                ps = kv_ps.tile([128, TQ], f32, name="ps_k", tag="ps_kv")
                for c in range(CCH):
                    nc.tensor.matmul(ps, wk_sb[:, c, ts(j, 128)],
                                     hT[:, 0:QT, c, :],
                                     start=(c == 0), stop=(c == CCH - 1))
                kt_t = stream.tile([128, TQ], fp8, name="kt_t", tag="kt_t", bufs=4)
                if trivial_b:
                    if on_act:
                        nc.scalar.activation(kt_t, ps, AF.Identity)
                    else:
                        nc.vector.tensor_copy(kt_t, ps)
                else:
                    nc.vector.tensor_scalar(out=kt_t, in0=ps,
                                            scalar1=bk_sb[:, j:j + 1],
                                            scalar2=None, op0=OP.add)
                nc.sync.dma_start(out=ko[a].ap()[ts(e, 128), :], in_=kt_t)
            nc.gpsimd.collective_compute(
                "AllGather", OP.bypass, replica_groups=GROUPS,
                ins=[ko[a].ap().opt()], outs=[kg[a].ap().opt()])
            for e in range(2):
                j = 2 * a + e
                ps = kv_ps.tile([128, TQ], f32, name="ps_q", tag="ps_kv")
                for c in range(CCH):
                    nc.tensor.matmul(ps, wq_sb[:, c, ts(j, 128)],
                                     hT[:, 0:QT, c, :],
                                     start=(c == 0), stop=(c == CCH - 1))
                if trivial_b:
                    if on_act:
                        nc.scalar.activation(qT[:, j], ps, AF.Identity)
                    else:
                        nc.vector.tensor_copy(qT[:, j], ps)
                else:
                    nc.vector.tensor_scalar(out=qT[:, j], in0=ps,
                                            scalar1=bq_sb[:, j:j + 1],
                                            scalar2=None, op0=OP.add)
            vos = vos_pool.tile([128, QT, 4 * VW], fp8, name="vos", tag="vos")
            vosv = vos.rearrange("p q (h x) -> p q h x", x=VW)
            nc.vector.memset(vosv[:, :, :, DH:DH + 1], 1.0)
            for i in range(QT):
                ps = kv_ps.tile([128, TQ], f32, name="ps_v", tag="ps_kv")
                for c in range(CCH):
                    nc.tensor.matmul(ps[:, 0:256], hT[:, i, c, :],
                                     wv_sb[:, c, ds(256 * a, 256)],
                                     start=(c == 0), stop=(c == CCH - 1))
                dst = vosv[:, i, :, 0:DH]
                psv = ps[:, 0:256].rearrange("p (h x) -> p h x", x=DH)
                if trivial_b:
                    if on_act:
                        nc.scalar.activation(dst, psv, AF.Identity)
                    else:
                        nc.vector.tensor_copy(dst, psv)
                else:
                    bvw = bv_bc[:, ds(256 * a, 256)].rearrange(
                        "p (h x) -> p h x", x=DH)
                    nc.vector.tensor_add(dst, psv, bvw)
            nc.sync.dma_start(
                out=vo[a].ap().rearrange("(q p) f -> p q f", p=128), in_=vos)
            nc.gpsimd.collective_compute(
                "AllGather", OP.bypass, replica_groups=GROUPS,
                ins=[vo[a].ap().opt()], outs=[vg[a].ap().opt()])

        def v_in(a):
            # gathered V chunk -> resident v_sb, per key tile
            for t in range(NT):
                r, w = t // 4, t % 4
                nc.sync.dma_start(
                    out=v_sb[:, t, ds(4 * VW * a, 4 * VW)],
                    in_=vg[a].ap()[r, ts(w, 128), :])

        # ---- attention (kT/V from the AllGathers, fp8 stationaries) ----
        scale = 1.0 / float(np.sqrt(DH))
        att_ctx = st.enter_context(ExitStack())
        s_ps = att_ctx.enter_context(tc.tile_pool(name="s_ps", bufs=2, space="PSUM"))
        y_ps = att_ctx.enter_context(tc.tile_pool(name="y_ps", bufs=1, space="PSUM"))
        kq_sb = att_ctx.enter_context(tc.tile_pool(name="kq_sb", bufs=2))
        att_sb = att_ctx.enter_context(tc.tile_pool(name="att_sb", bufs=3))
        resc = {}

        def attention_main(j):
            a, e = j // 2, j % 2
            kT_j = kq_sb.tile([128, T], fp8, name="kT_j", tag="kT_j")
            for r in range(GROUP):
                nc.sync.dma_start(out=kT_j[:, ds(TQ * r, TQ)],
                                  in_=kg[a].ap()[r, ts(e, 128), :])
            ps_y1 = y_ps.tile([VW, 512], f32, name="ps_y1", tag="ps_y1")
            ps_y2 = y_ps.tile([VW, 512], f32, name="ps_y2", tag="ps_y2")
            for cidx in range(NT):
                ps_s = s_ps.tile([128, 1024], f32, name="ps_s", tag="ps_s")
                nc.tensor.matmul(ps_s[:, 0:512],
                                 kT_j[0:64, ts(cidx, 128)],
                                 qT[0:64, j, :], start=True, stop=True)
                nc.tensor.matmul(ps_s[:, 512:1024],
                                 kT_j[64:128, ts(cidx, 128)],
                                 qT[64:128, j, :], start=True, stop=True,
                                 tile_position=(64, 0))
                pT = att_sb.tile([128, 1024], bf16, name="pT", tag="pT")
                nc.scalar.activation(pT, ps_s, AF.Exp, scale=scale)
                nc.tensor.matmul(ps_y1,
                                 v_sb[:, cidx, ds(VW * 2 * j, VW)],
                                 pT[:, 0:512],
                                 start=(cidx == 0), stop=(cidx == NT - 1))
                nc.tensor.matmul(ps_y2,
                                 v_sb[:, cidx, ds(VW * (2 * j + 1), VW)],
                                 pT[:, 512:1024],
                                 start=(cidx == 0), stop=(cidx == NT - 1))
            # copy Y and the sums row out of PSUM right away so the
            # accumulator banks free up for the next pair; the sums
            # staging copy also moves them to SBUF partition 0
            # (custom-DVE ops mis-read PSUM at a partition offset)
            for u, ps_y in ((0, ps_y1), (1, ps_y2)):
                ycp = att_sb.tile([64, 512], f32, name="ycp", tag=f"ycp{u}", bufs=2)
                nc.vector.tensor_copy(ycp, ps_y[0:DH, :])
                rs0 = att_sb.tile([1, 512], f32, name="rs0", tag=f"rs0{u}", bufs=2)
                nc.vector.tensor_copy(rs0, ps_y[DH:DH + 1, :])
                rs = att_sb.tile([1, 512], f32, name="rs", tag=f"rs{u}", bufs=2)
                nc.vector.reciprocal_approx_fast(rs, rs0)
                resc[(j, u)] = (ycp, rs)

        def attention_rescale(j):
            # partition_broadcast lives on gpsimd: emitted one chunk late so
            # every AllGather trigger is already queued ahead of it
            for u in range(2):
                ycp, rs = resc.pop((j, u))
                bc = att_sb.tile([64, 512], f32, name="bc", tag="bc")
                nc.gpsimd.partition_broadcast(bc, rs)
                nc.vector.tensor_mul(ynT[64 * u:64 * u + 64, j, :], ycp, bc)

        # proj/MLP-era constants: emitted early so their DMAs run during
        # the attention phase
        wp_sb = pR.tile([128, CCH, C], bf16)
        nc.sync.dma_start(out=wp_sb, in_=wp.ap())
        if not trivial_ln2:
            ln2w_bc = pR.tile([128, C], f32)
            nc.sync.dma_start(out=ln2w_bc, in_=_bcast(ln2w.ap()))
            ln2b_bc = pR.tile([128, C], f32)
            nc.sync.dma_start(out=ln2b_bc, in_=_bcast(ln2b.ap()))
        else:
            ln2w_bc = ln2b_bc = None
        if not trivial_b:
            bp_bc = pR.tile([128, C], f32)
            nc.sync.dma_start(out=bp_bc, in_=_bcast(bp.ap()))
        bfc_sb = pR.tile([128, FT], f32)
        nc.sync.dma_start(out=bfc_sb, in_=bf_.ap())
        bm_bc = pR.tile([128, C], f32)
        nc.sync.dma_start(out=bm_bc, in_=_bcast(bm.ap()))
        ident = pR.tile([128, 128], f32)
        nc.sync.dma_start(out=ident, in_=idw.ap())

        chunk(0, True)
        v_in(0)
        attention_main(0)
        chunk(1, False)
        v_in(1)
        attention_rescale(0)
        attention_main(1)
        chunk(2, False)
        v_in(2)
        attention_rescale(1)
        attention_main(2)
        chunk(3, False)
        v_in(3)
        attention_rescale(2)
        attention_main(3)
        attention_rescale(3)
        for j in range(4, PAIRS):
            attention_main(j)
            attention_rescale(j)
        att_ctx.close()
        kvst.close()
        stA.close()

        # ---- attn projection + residual + LN2 + h2^T ----
        pD = st.enter_context(tc.tile_pool(name="pD", bufs=1, side="left"))
        x2 = pD.tile([128, QT, C], f32)
        h2T = pD.tile([128, QT, CCH, 128], bf16)

        with tc.tile_pool(name="ap_ps", bufs=2, space="PSUM") as ap_ps:
            for i in range(QT):
                if not trivial_b:
                    nc.vector.tensor_add(xqs[:, i], xqs[:, i], bp_bc)
                for n in range(C // 512):
                    ps = ap_ps.tile([128, 512], f32, name="ps_a", tag="ps_a")
                    for j in range(PAIRS):
                        nc.tensor.matmul(ps, ynT[:, j, ts(i, 128)],
                                         wp_sb[:, j, ds(512 * n, 512)],
                                         start=(j == 0), stop=(j == PAIRS - 1))
                    nc.vector.tensor_add(x2[:, i, ds(512 * n, 512)], ps,
                                         xqs[:, i, ds(512 * n, 512)])
                h2_t = stream.tile([128, C], bf16, name="h2_t", tag="h_t", bufs=6)
                layer_norm(x2[:, i, :], ln2w_bc, ln2b_bc, h2_t, trivial_ln2)
                nc.sync.dma_start_transpose(h2T[:, i], h2_t[:])
                if i == QT - 1:
                    # prefetch the gelu table while proj/LN2 finish; input is
                    # the LN2 output tile so the scheduler cannot hoist it
                    # before the LN rstds (they share the ACT queue)
                    gd = stat.tile([128, 1], f32, name="gd", bufs=6)
                    nc.scalar.activation(gd, h2_t[:, 0:1], AF.Gelu_apprx_tanh)

        # ---- MLP ----
        # fold the mlp_proj bias into the residual copy while fc runs
        for i in range(QT):
            nc.vector.tensor_add(x2[:, i, :], x2[:, i, :], bm_bc)
        gT = pD.tile([128, FT, TQ], bf16)
        with tc.tile_pool(name="fc_ps", bufs=4, space="PSUM") as fc_ps, \
             tc.tile_pool(name="wf_sb", bufs=6) as wf_pool:
            wf_tiles = {}
            for t in range(6):
                wf_tiles[t] = wf_pool.tile([128, CCH, 128], bf16,
                                           name="wf_t", tag="wf_t")
                nc.sync.dma_start(out=wf_tiles[t], in_=wf.ap()[t])
            for t in range(FT):
                if t in wf_tiles:
                    wf_t = wf_tiles.pop(t)
                else:
                    wf_t = wf_pool.tile([128, CCH, 128], bf16, name="wf_t",
                                        tag="wf_t")
                    nc.sync.dma_start(out=wf_t, in_=wf.ap()[t])
                ps = fc_ps.tile([128, 512], f32, name="ps_f", tag="ps_f")
                for c in range(CCH):
                    nc.tensor.matmul(ps, wf_t[:, c, :], h2T[:, 0:QT, c, :],
                                     start=(c == 0), stop=(c == CCH - 1))
                nc.scalar.activation(gT[:, t, :], ps, AF.Gelu_apprx_tanh,
                                     bias=bfc_sb[:, t:t + 1], scale=1.0)

        with tc.tile_pool(name="m_ps", bufs=1, space="PSUM") as m_ps, \
             tc.tile_pool(name="wm_sb", bufs=6) as wm_pool, \
             tc.tile_pool(name="out_sb", bufs=2) as out_pool:
            ps_m = [m_ps.tile([128, 512], f32, name=f"ps_m{k}", tag=f"ps_m{k}")
                    for k in range(8)]
            for i in range(QT):
                nc.tensor.matmul(ps_m[i * 2 + 1], ident,
                                 x2[:, i, ds(512, 512)],
                                 start=True, stop=False)
            for t in range(FT):
                wm_t = wm_pool.tile([128, C], bf16, name="wm_t", tag="wm_t")
                nc.sync.dma_start(out=wm_t, in_=wm.ap()[ts(t, 128), :])
                for i in range(QT):
                    for n in range(C // 512):
                        nc.tensor.matmul(ps_m[i * 2 + n], gT[:, t, ts(i, 128)],
                                         wm_t[:, ds(512 * n, 512)],
                                         start=(t == 0 and n == 0),
                                         stop=(t == FT - 1))
            for i in range(QT):
                out_t = out_pool.tile([128, C], f32, name="out_t", tag="out_t")
                # n==0: residual added here on DVE; n==1: residual was
                # preloaded into PSUM, a scalar-engine copy suffices -- the
                # two run concurrently and each chunk DMAs out immediately
                nc.vector.tensor_add(out_t[:, 0:512], ps_m[i * 2],
                                     x2[:, i, 0:512])
                nc.sync.dma_start(out=out.ap()[ts(i, 128), 0:512],
                                  in_=out_t[:, 0:512])
                nc.scalar.activation(out_t[:, 512:1024], ps_m[i * 2 + 1],
                                     AF.Identity)
                nc.sync.dma_start(out=out.ap()[ts(i, 128), ds(512, 512)],
                                  in_=out_t[:, 512:1024])


def _get_program(trivial_ln1, trivial_ln2, trivial_b):
    key = (trivial_ln1, trivial_ln2, trivial_b)
    if key not in _CACHED:
        _CACHED[key] = _build_program(trivial_ln1, trivial_ln2, trivial_b)
    return _CACHED[key]


def _tile_proj_weight(w):
    # [C, N] f32 -> [128, CCH, N] bf16 with partition = c % 128, chunk = c // 128
    w = np.asarray(w, np.float32).reshape(CCH, 128, -1)
    return np.ascontiguousarray(w.transpose(1, 0, 2).astype(ml_dtypes.bfloat16))


def _prep_in_maps(inputs):
    fl = lambda a: np.ascontiguousarray(np.asarray(a, np.float32))
    x = fl(inputs["x"])
    attn_w = fl(inputs["attn_w"])
    attn_b = fl(inputs["attn_b"])
    wf_full = fl(inputs["fc_w"])  # [C, F]
    # wf tiled: [FT, 128(c), CCH, 128(f')]
    wf_t = wf_full.reshape(CCH, 128, FT, 128).transpose(2, 1, 0, 3)
    wf_t = np.ascontiguousarray(wf_t.astype(ml_dtypes.bfloat16))
    pb = lambda b: np.ascontiguousarray(
        np.asarray(b, np.float32).reshape(-1, 128).T)  # [128, tiles]
    shared = {
        "wq": _tile_proj_weight(attn_w[:, 0:C]),
        "wk": _tile_proj_weight(attn_w[:, C:2 * C]),
        "wv": _tile_proj_weight(attn_w[:, 2 * C:3 * C]),
        "bq": pb(attn_b[0:C]), "bk": pb(attn_b[C:2 * C]),
        "bv": fl(attn_b[2 * C:3 * C]),
        "ln1w": fl(inputs["ln1_w"]), "ln1b": fl(inputs["ln1_b"]),
        "ln2w": fl(inputs["ln2_w"]), "ln2b": fl(inputs["ln2_b"]),
        "wp": _tile_proj_weight(inputs["attn_proj_w"]),
        "bp": fl(inputs["attn_proj_b"]),
        "wf": wf_t, "bf": pb(inputs["fc_b"]),
        "wm": np.ascontiguousarray(fl(inputs["mlp_proj_w"]).astype(ml_dtypes.bfloat16)),
        "bm": fl(inputs["mlp_proj_b"]),
        "idw": np.eye(128, dtype=np.float32),
    }
    in_maps = []
    for core in range(NCORES):
        b, r = core // GROUP, core % GROUP
        in_maps.append({
            "xq": np.ascontiguousarray(x[b, TQ * r:TQ * (r + 1)]),
            **shared,
        })
    return in_maps


def run(inputs, trace=False):
    trivial_ln1 = bool(np.all(np.asarray(inputs["ln1_w"]) == 1.0)
                       and np.all(np.asarray(inputs["ln1_b"]) == 0.0))
    trivial_ln2 = bool(np.all(np.asarray(inputs["ln2_w"]) == 1.0)
                       and np.all(np.asarray(inputs["ln2_b"]) == 0.0))
    trivial_b = bool(np.all(np.asarray(inputs["attn_b"]) == 0.0))
    nc = _get_program(trivial_ln1, trivial_ln2, trivial_b)
    in_maps = _prep_in_maps(inputs)
    res = run_bass_kernel_spmd(nc, in_maps, core_ids=list(range(NCORES)),
                               trace=trace)
    out = np.empty((B, T, C), np.float32)
    for core in range(NCORES):
        b, r = core // GROUP, core % GROUP
        out[b, TQ * r:TQ * (r + 1)] = res.results[core]["out"]
    return out, res


def kernel(**inputs):
    out, _ = run(inputs, trace=False)
    return out
